# revision 4
# baseline (speedup 1.0000x reference)
"""Trainium2 Bass kernel for nn_AttnResBlock (B=64, CH1=3, CH2=4, HID=16, T=16384, E=512).

Strategy: tensor-parallel split of the T dimension across 8 cores, fp16
datapath for every large tensor.

  y = p + W3 @ (attn(W1@p, W2@c | Wq,Wk,Wv) @ Wo)        p = x[:,:3], c = x[:,3:7]

The big weights (Wq/Wk/Wv [E,T], Wo [T,E]) dominate memory traffic.  Each
core owns a T-slice of 2048 and reads only its slice of each projection
weight; all large streams are fp16 (host-rounded), halving HBM bytes vs
fp32 while the PE accumulates in fp32 PSUM (matmul error ~1e-3 rel, well
inside the 2e-2 gate).

  stage A (T-parallel):   P[b,c,e]     = sum_{t in Ti} p[b,c,t] Wq[e,t]   (partial)
                          Ck/Cv[b,c,e] = sum_{t in Ti} c[b,c,t] W{k,v}[e,t]
    11 rows/batch (3 P + 4 Ck + 4 Cv) go through a fp16 ReduceScatter;
    core r receives batches 8r..8r+7.  Channel mixes and biases are applied
    AFTER the reduce (biases via a precomputed [49,E] bias plane added by
    DVE, so they ride along with the PSUM->SBUF eviction).
  stage B (B-parallel):   two block-diag matmuls per batch produce Q [16,E]
                          and K/(W3@W2 V-mix) [36,E] (W3 is folded into the
                          V path on the host, so attention directly emits
                          the 3 W3-mixed channels; the W3V block sits at
                          partition base 32 so its PE transposes are
                          tile-position legal; transpose col 3 = 1.0 from
                          the bias plane makes the attn matmul also emit
                          the softmax sums Z as row 3); S^T = K^T Q,
                          exp(S*scale - 10) on ACT (fp16 out, shift keeps
                          exp in fp16 range; Z divides it out exactly),
                          attn = (W3V | 1)^T @ exp.  Unnormalized att3+Z
                          in [e,(b,[o,Z])] layout is AllGather'ed
                          (tiny: 32KB/core); normalization happens in
                          stage C where the DVE is idle.
  stage C (T-parallel):   reciprocal+broadcast-multiply normalize, then
                          y^T[(b,o), t] = at3^T Wo-slice contraction over
                          e, + bo*w3sum rank-1 term, + residual p (fp16
                          xp, loaded during the collective window via
                          eviction-buffer reuse WAR gating).

DMA instruction count is kept low (HWDGE dispatch costs ~625ns/DMA and is
single-slot): x and a host-concatenated [wq|wk|wv] stream in k-tile chunks
(small chunks first so the PE starts early), the ReduceScatter staging is
6 DMAs with the (ch,b)->(b,ch) regroup done by a 3-d DRAM-side access
pattern (SBUF side stays a plain 2-d partition walk; dma_start only
requires equal sizes), and stage B reads all post-scatter rows in one DMA.
"""

import numpy as np

import concourse.bacc as bacc
import concourse.tile as tile
import concourse.mybir as mybir

F32 = mybir.dt.float32
F16 = mybir.dt.float16
AF = mybir.ActivationFunctionType
ALU = mybir.AluOpType

B, CH1, CH2, HID, T, E = 64, 3, 4, 16, 16384, 512
NCORES = 8
TLOC = T // NCORES          # 2048
KT = TLOC // 128            # 16 k-tiles in stage A
KCH = 4                     # k-tiles per DMA chunk
NCHUNK = KT // KCH          # 8 chunks
EC = E // 128               # 4 e/f chunks
BLOC = B // NCORES          # 8 batches per core in stage B
NCH = CH1 + CH2 + CH2       # 11 rows/batch through the ReduceScatter
NKV = 2 * HID + CH1 + 1     # 36 KV-mix rows: K 0..15, pad, W3V 32..34, Z 35
SCALE = 1.0 / np.sqrt(HID)  # attention scale
EXPSHIFT = 10.0             # exp(S*SCALE - 10): keeps exp inside fp16 range;
                            # softmax is shift-invariant so Z divides it out
CH1Z = CH1 + 1              # 3 W3-mixed channels + the softmax sum Z
GW = CH1Z * BLOC            # 32 AllGather columns per rank: (b, [o0 o1 o2 Z])


def build_program(reps: int = 1, dbg: bool = False):
    nc = bacc.Bacc("TRN2", target_bir_lowering=False, debug=False,
                   num_devices=NCORES)

    xt = nc.dram_tensor("xt", [TLOC, 7 * B], F16, kind="ExternalInput")
    xp = nc.dram_tensor("xp", [CH1 * B, TLOC], F16, kind="ExternalInput")
    wqkv = nc.dram_tensor("wqkv", [TLOC, 3 * E], F16, kind="ExternalInput")
    wot = nc.dram_tensor("wot", [E, TLOC], F16, kind="ExternalInput")
    bot = nc.dram_tensor("bot", [1, TLOC], F16, kind="ExternalInput")
    w12q = nc.dram_tensor("w12q", [NCH, HID], F16, kind="ExternalInput")
    w12kv = nc.dram_tensor("w12kv", [NCH, NKV], F16, kind="ExternalInput")
    biasqkv = nc.dram_tensor("biasqkv", [NKV, 2 * E], F32,
                             kind="ExternalInput")
    identt = nc.dram_tensor("identt", [NKV, CH1Z], F16,
                            kind="ExternalInput")
    w3sr = nc.dram_tensor("w3sr", [1, CH1 * B], F16, kind="ExternalInput")
    yt = nc.dram_tensor("yt", [CH1 * B, TLOC], F32, kind="ExternalOutput")

    rg = [list(range(NCORES))]

    with tile.TileContext(nc) as tc:
        with tc.tile_pool(name="const", bufs=1) as cp, \
             tc.tile_pool(name="dram", space="DRAM", bufs=1) as dp, \
             tc.tile_pool(name="xp", bufs=1) as xpool, \
             tc.tile_pool(name="wotp", bufs=1) as wotp:

            # small constants via the Pool (SWDGE) queue so they never
            # contend with the stage-A weight stream on HWDGE
            w12q_s = cp.tile([NCH, HID], F16, name="w12q_s")
            w12kv_s = cp.tile([NCH, NKV], F16, name="w12kv_s")
            biasqkv_s = cp.tile([NKV, 2 * E], F32, name="biasqkv_s")
            identt_s = cp.tile([NKV, CH1Z], F16, name="identt_s")
            w3sr_s = cp.tile([1, CH1 * B], F16, name="w3sr_s")
            bot_s = cp.tile([1, TLOC], F16, name="bot_s")
            nshift = cp.tile([128, 1], F32, name="nshift")
            nc.vector.memset(nshift[:], -EXPSHIFT)
            nc.gpsimd.dma_start(w12q_s[:], w12q[:])
            nc.gpsimd.dma_start(w12kv_s[:], w12kv[:])
            nc.gpsimd.dma_start(biasqkv_s[:], biasqkv[:])
            nc.gpsimd.dma_start(identt_s[:], identt[:])
            nc.gpsimd.dma_start(w3sr_s[:], w3sr[:])
            nc.gpsimd.dma_start(bot_s[:], bot[:])

            dbg_outs = {}
            if dbg:
                dbg_outs["dbg_rs"] = nc.dram_tensor(
                    "dbg_rs", [BLOC * NCH, E], F16, kind="ExternalOutput")
                dbg_outs["dbg_rsin"] = nc.dram_tensor(
                    "dbg_rsin", [B * NCH, E], F16, kind="ExternalOutput")
                dbg_outs["dbg_q"] = nc.dram_tensor(
                    "dbg_q", [HID, E], F16, kind="ExternalOutput")
                dbg_outs["dbg_kv"] = nc.dram_tensor(
                    "dbg_kv", [NKV, E], F16, kind="ExternalOutput")
                dbg_outs["dbg_et"] = nc.dram_tensor(
                    "dbg_et", [128, EC * E], F16, kind="ExternalOutput")
                dbg_outs["dbg_ag"] = nc.dram_tensor(
                    "dbg_ag", [NCORES * E, GW], F16, kind="ExternalOutput")
            for rep in range(reps):
                build_rep(nc, tc, dp, xpool, wotp, rep, rg, locals())

    nc.compile()
    return nc


def build_rep(nc, tc, dp, xpool, wotp, rep, rg, env):
    xt, xp, wqkv, wot, yt = (env[k] for k in
                             ("xt", "xp", "wqkv", "wot", "yt"))
    w12q_s, w12kv_s, biasqkv_s, identt_s = (
        env[k] for k in ("w12q_s", "w12kv_s", "biasqkv_s", "identt_s"))
    w3sr_s, bot_s, nshift = (env[k] for k in ("w3sr_s", "bot_s", "nshift"))
    dbg_outs = env.get("dbg_outs", {})

    r = f"r{rep}"

    # ---- DRAM bounce buffers for the collectives -------------------------
    rs_in = dp.tile([B * NCH, E], F16, name=f"rs_in_{r}", tag="rs_in", bufs=1)
    rs_out = dp.tile([BLOC * NCH, E], F16, name=f"rs_out_{r}", tag="rs_out",
                     bufs=1)
    ag_in = dp.tile([E, GW], F16, name=f"ag_in_{r}", tag="ag_in", bufs=1)
    ag_out = dp.tile([NCORES * E, GW], F16, name=f"ag_out_{r}", tag="ag_out",
                     bufs=1)

    # ---- stage A: big T-contraction ------------------------------------
    # x and the host-concatenated [wq|wk|wv] stream in k-tile chunks (two
    # HWDGE dispatches per chunk); small chunks first so the PE starts fast
    CHUNKS = (1, 1, 1, 1, 2, 2, 4, 4)
    assert sum(CHUNKS) == KT
    xts, wws = [], []
    k0c = 0
    for c, kch in enumerate(CHUNKS):
        xc = xpool.tile([128, kch * 7 * B], F16, name=f"x{c}_{r}",
                        tag=f"x{c}", bufs=1)
        wc = xpool.tile([128, kch * 3 * E], F16, name=f"ww{c}_{r}",
                        tag=f"ww{c}", bufs=1)
        rows = slice(k0c * 128, (k0c + kch) * 128)
        nc.sync.dma_start(
            xc[:].rearrange("p (a w) -> p a w", a=kch),
            xt[rows, :].rearrange("(a p) w -> p a w", a=kch))
        nc.sync.dma_start(
            wc[:].rearrange("p (a e) -> p a e", a=kch),
            wqkv[rows, :].rearrange("(a p) e -> p a e", a=kch))
        xts.append(xc)
        wws.append(wc)
        k0c += kch

    with tc.tile_pool(name="psA", space="PSUM", bufs=1) as psA, \
         tc.tile_pool(name="stgA", bufs=1) as sa:
        p0 = psA.tile([128, E], F32, name=f"p0_{r}", tag="p0", bufs=1)
        p1 = psA.tile([64, E], F32, name=f"p1_{r}", tag="p1", bufs=1)
        k0 = psA.tile([128, E], F32, name=f"k0_{r}", tag="k0", bufs=1)
        k1 = psA.tile([128, E], F32, name=f"k1_{r}", tag="k1", bufs=1)
        v0 = psA.tile([128, E], F32, name=f"v0_{r}", tag="v0", bufs=1)
        v1 = psA.tile([128, E], F32, name=f"v1_{r}", tag="v1", bufs=1)

        k0c = 0
        for c, kch in enumerate(CHUNKS):
            xc, wc = xts[c], wws[c]
            for a in range(kch):
                k = k0c + a
                st, sp = (k == 0), (k == KT - 1)
                x0 = a * 7 * B
                w0 = a * 3 * E
                wq_a = wc[:, w0:w0 + E]
                wk_a = wc[:, w0 + E:w0 + 2 * E]
                wv_a = wc[:, w0 + 2 * E:w0 + 3 * E]
                nc.tensor.matmul(p0[:], xc[:, x0:x0 + 128], wq_a,
                                 start=st, stop=sp)
                nc.tensor.matmul(p1[:], xc[:, x0 + 128:x0 + 192], wq_a,
                                 start=st, stop=sp)
                nc.tensor.matmul(k0[:], xc[:, x0 + 192:x0 + 320], wk_a,
                                 start=st, stop=sp)
                nc.tensor.matmul(v0[:], xc[:, x0 + 192:x0 + 320], wv_a,
                                 start=st, stop=sp)
                nc.tensor.matmul(k1[:], xc[:, x0 + 320:x0 + 448], wk_a,
                                 start=st, stop=sp)
                nc.tensor.matmul(v1[:], xc[:, x0 + 320:x0 + 448], wv_a,
                                 start=st, stop=sp)
            k0c += kch

        # PSUM -> SBUF fp16, split across the two PSUM-capable engines.
        # p0s/k0s/v0s live in the rep-long xpool, sized [128, 2*TLOC] (only
        # [:, :E] used) so the wot/xpall tiles below reuse their buffers:
        # the tag-rotation WAR dependency delays those big loads until the
        # staging DMAs have read the evictions — i.e. into the collective
        # window — instead of stealing stage-A DMA bandwidth.
        p0s = xpool.tile([128, 2 * TLOC], F16, name=f"p0s_{r}", tag="p0s",
                         bufs=1)
        p1s = sa.tile([64, E], F16, name=f"p1s_{r}", tag="p1s", bufs=1)
        k0s = xpool.tile([128, 2 * TLOC], F16, name=f"k0s_{r}", tag="k0s",
                         bufs=1)
        k1s = sa.tile([128, E], F16, name=f"k1s_{r}", tag="k1s", bufs=1)
        v0s = xpool.tile([128, 2 * TLOC], F16, name=f"v0s_{r}", tag="v0s",
                         bufs=1)
        v1s = sa.tile([128, E], F16, name=f"v1s_{r}", tag="v1s", bufs=1)
        nc.vector.tensor_copy(p0s[:, 0:E], p0[:])
        nc.scalar.activation(p1s[:], p1[:], AF.Copy)
        nc.vector.tensor_copy(k0s[:, 0:E], k0[:])
        nc.scalar.activation(k1s[:], k1[:], AF.Copy)
        nc.vector.tensor_copy(v0s[:, 0:E], v0[:])
        nc.scalar.activation(v1s[:], v1[:], AF.Copy)

        # (ch,b) rows -> rs_in's (b,ch) rows, regrouped by the DMA pattern.
        # ch layout per batch: [P0 P1 P2 | Ck0..3 | Cv0..3].  The SBUF side
        # stays a plain 2-d partition walk (same element order as the 3-d
        # DRAM view; dma_start only requires equal sizes).
        rs_v = rs_in[:].rearrange("(b c) e -> c b e", c=NCH)
        for s_t, ch0, nch in ((p0s, 0, 2), (p1s, 2, 1), (k0s, 3, 2),
                              (k1s, 5, 2), (v0s, 7, 2), (v1s, 9, 2)):
            nc.sync.dma_start(rs_v[ch0:ch0 + nch], s_t[:, 0:E])

    nc.gpsimd.collective_compute(
        "ReduceScatter", ALU.add, replica_groups=rg,
        ins=[rs_in.opt()], outs=[rs_out.opt()],
    )
    if rep == 0 and "dbg_rs" in dbg_outs:
        nc.gpsimd.dma_start(dbg_outs["dbg_rs"][:], rs_out[:])
        nc.gpsimd.dma_start(dbg_outs["dbg_rsin"][:], rs_in[:])

    # all post-scatter rows in one DMA, channel-major so every per-batch
    # slice starts at partition 0 (engines cannot shift partition lanes);
    # emitted BEFORE the wot/xpall loads so it dispatches the moment the
    # ReduceScatter completes instead of queueing behind them
    rs_all = xpool.tile([NCH, BLOC * E], F16, name=f"rs_all_{r}",
                        tag="rs_all", bufs=1)
    nc.sync.dma_start(
        rs_all[:].rearrange("c (b e) -> c b e", b=BLOC),
        rs_out[:].rearrange("(b c) e -> c b e", c=NCH))

    # stage-C inputs, buffer-reusing the eviction tags (see above): the
    # WAR dependency releases them at the stage-A tail, so they stream
    # during the collective window
    wots = []
    for half, tag in ((0, "p0s"), (1, "k0s")):
        wo_t = xpool.tile([128, 2 * TLOC], F16, name=f"wo{half}_{r}",
                          tag=tag, bufs=1)
        nc.sync.dma_start(
            wo_t[:].rearrange("p (a t) -> p a t", a=2),
            wot[half * 256:(half + 1) * 256, :].rearrange(
                "(a p) t -> p a t", a=2))
        wots.append(wo_t)
    xpbig = xpool.tile([128, 2 * TLOC], F16, name=f"xpall_{r}", tag="v0s",
                       bufs=1)
    xpall = xpbig[0:96, :]
    nc.sync.dma_start(
        xpall.rearrange("p (m t) -> p m t", m=2),
        xp[:].rearrange("(m p) t -> p m t", m=2))

    # ---- stage B: per-batch attention ----------------------------------
    with tc.tile_pool(name="psB", space="PSUM", bufs=1) as psB, \
         tc.tile_pool(name="sbB", bufs=2) as sb:
        ag_s = sb.tile([128, EC * GW], F16, name=f"ag_s_{r}", tag="ag_s",
                       bufs=1)

        # per-batch state carried across the 1-batch software-pipeline skew
        qkv_sL = [None] * BLOC
        vt_sL = [None] * BLOC
        etL = [None] * BLOC
        apsL = [None] * BLOC

        def emit_mix(b):
            """channel mixes for batch b: Q and K/W3V/Z in separate PSUM
            tiles so the two DVE evictions (which also apply the biases)
            free their banks independently and pipeline with the consumers.
            qkv_s layout: columns 0..E-1 = Q (rows 0..15), E..2E-1 = K
            (rows 0..15) / W3V+ones (rows 32..35)."""
            rs_b = rs_all[:, b * E:(b + 1) * E]
            q_ps = psB.tile([HID, E], F32, name=f"qps{b}_{r}", tag="qps",
                            bufs=1)
            kv_ps = psB.tile([NKV, E], F32, name=f"kvps{b}_{r}", tag="kvps",
                             bufs=1)
            nc.tensor.matmul(q_ps[:], w12q_s[:], rs_b, start=True, stop=True)
            nc.tensor.matmul(kv_ps[:], w12kv_s[:], rs_b, start=True,
                             stop=True)
            qkv_s = sb.tile([NKV, 2 * E], F16, name=f"qkv_s{b}_{r}",
                            tag="qkv_s", bufs=2)
            nc.vector.tensor_tensor(out=qkv_s[0:HID, 0:E], in0=q_ps[:],
                                    in1=biasqkv_s[0:HID, 0:E], op=ALU.add)
            nc.vector.tensor_tensor(out=qkv_s[:, E:2 * E], in0=kv_ps[:],
                                    in1=biasqkv_s[:, E:2 * E], op=ALU.add)
            qkv_sL[b] = qkv_s

        def emit_vt(b):
            """(W3V | 1)^T: rows 32..35 of the K-half -> [128, 4] per
            f-chunk.  vtp and atp share PSUM banks (tag "tp")."""
            qkv_s = qkv_sL[b]
            vtp = psB.tile([128, EC * CH1Z], F16, name=f"vtp{b}_{r}",
                           tag="tp", bufs=1)
            for fc in range(EC):
                nc.tensor.transpose(
                    vtp[:, fc * CH1Z:(fc + 1) * CH1Z],
                    qkv_s[2 * HID:NKV, E + fc * 128:E + (fc + 1) * 128],
                    identt_s[2 * HID:NKV, :])
            vt_s = sb.tile([128, EC * CH1Z], F16, name=f"vt_s{b}_{r}",
                           tag="vt_s", bufs=2)
            nc.vector.tensor_copy(vt_s[:], vtp[:])
            vt_sL[b] = vt_s

        def emit_s(b):
            """four S^T chunk matmuls, one exp each (chunk granularity
            keeps the PE->ACT pipeline fine-grained)."""
            qkv_s = qkv_sL[b]
            q_ap = qkv_s[0:HID, 0:E]
            ets = []
            for fc in range(EC):
                sps = psB.tile([128, E], F32, name=f"sps{b}{fc}_{r}",
                               tag="s", bufs=2)
                nc.tensor.matmul(
                    sps[:], qkv_s[0:HID, E + fc * 128:E + (fc + 1) * 128],
                    q_ap, start=True, stop=True)
                et = sb.tile([128, E], F16, name=f"et{b}{fc}_{r}", tag="et",
                             bufs=8)
                nc.scalar.activation(et[:], sps[:], AF.Exp, scale=SCALE,
                                     bias=nshift[:])
                ets.append(et)
            etL[b] = ets

        def emit_attn(b):
            """attn matmuls for batch b (needs et[b] ready).

            Rows: 0..2 = W3-mixed attention (unnormalized), 3 = Z."""
            aps = psB.tile([CH1Z, E], F32, name=f"aps{b}_{r}", tag="attn",
                           bufs=2)
            vt_s, ets = vt_sL[b], etL[b]
            for fc in range(EC):
                nc.tensor.matmul(aps[:], vt_s[:, fc * CH1Z:(fc + 1) * CH1Z],
                                 ets[fc][:], start=(fc == 0), stop=(fc == 3))
            apsL[b] = aps

        def emit_back(b):
            """transpose attn rows into e-partition layout for the gather."""
            aps = apsL[b]
            an_s = sb.tile([CH1Z, E], F16, name=f"an_s{b}_{r}", tag="an_s",
                           bufs=2)
            nc.vector.tensor_copy(an_s[:], aps[:])
            atp = psB.tile([128, EC * CH1Z], F16, name=f"atp{b}_{r}",
                           tag="atp", bufs=1)
            for ec in range(EC):
                nc.tensor.transpose(
                    atp[:, ec * CH1Z:(ec + 1) * CH1Z],
                    an_s[:, ec * 128:(ec + 1) * 128],
                    identt_s[0:CH1Z, :])
            # ag_s column layout per e-chunk block: (b, [o0 o1 o2 Z])
            nc.vector.tensor_copy(
                ag_s[:].rearrange("p (c q) -> p c q", c=EC)[
                    :, :, b * CH1Z:(b + 1) * CH1Z],
                atp[:].rearrange("p (c k) -> p c k", c=EC))

        # software pipeline with a 1-batch skew: after each mix, the PE
        # chews on batch b-1 (attn + output transposes) while the DVE
        # evicts batch b's mixes, so no engine waits on the
        # mix->evict->S chain
        emit_mix(0)
        emit_vt(0)
        emit_s(0)
        for b in range(1, BLOC):
            emit_mix(b)
            emit_attn(b - 1)
            emit_back(b - 1)
            emit_vt(b)
            emit_s(b)
        emit_attn(BLOC - 1)
        emit_back(BLOC - 1)

        if rep == 0 and "dbg_q" in dbg_outs:
            nc.gpsimd.dma_start(dbg_outs["dbg_q"][:], qkv_sL[0][0:HID, 0:E])
            nc.gpsimd.dma_start(dbg_outs["dbg_kv"][:],
                                qkv_sL[0][:, E:2 * E])
        if rep == 0 and "dbg_et" in dbg_outs:
            nc.gpsimd.dma_start(dbg_outs["dbg_et"][:], etL[0][:])

        nc.sync.dma_start(
            ag_in[:].rearrange("(c p) w -> p c w", c=EC),
            ag_s[:].rearrange("p (c w) -> p c w", c=EC))

    nc.gpsimd.collective_compute(
        "AllGather", ALU.bypass, replica_groups=rg,
        ins=[ag_in.opt()], outs=[ag_out.opt()],
    )
    if rep == 0 and "dbg_ag" in dbg_outs:
        nc.gpsimd.dma_start(dbg_outs["dbg_ag"][:], ag_out[:])

    # ---- stage C: y^T[(b,o), t] = at3^T Wo^T + bias + residual ----------
    from concourse.bass import broadcast_tensor_aps

    with tc.tile_pool(name="psC", space="PSUM", bufs=1) as psC, \
         tc.tile_pool(name="sbC", bufs=1) as sc2:
        MH = CH1 * B // 2      # 96 (b,o) rows per M-half
        atall = sc2.tile([128, EC * NCORES * GW], F16, name=f"atall_{r}",
                         tag="atall", bufs=1)
        ag_v = ag_out[:].rearrange("(g c p) w -> c p g w", g=NCORES, c=EC)
        for ec in range(EC):
            nc.gpsimd.dma_start(
                atall[:, ec * NCORES * GW:(ec + 1) * NCORES * GW].rearrange(
                    "p (g w) -> p g w", g=NCORES),
                ag_v[ec])
        at3 = []
        for ec in range(EC):
            a_u = atall[:, ec * NCORES * GW:(ec + 1) * NCORES * GW]
            # normalize: at3[e, (g,b,o)] = att[e,(g,b,o)] * (1/Z[e,(g,b)])
            a_n = sc2.tile([128, CH1 * B], F16, name=f"at3{ec}_{r}",
                           tag=f"at3{ec}", bufs=1)
            u = a_u.rearrange("p (g b k) -> p g b k", g=NCORES, b=BLOC)
            zr = sc2.tile([128, B], F32, name=f"zr{ec}_{r}", tag=f"zr{ec}",
                          bufs=1)
            zr_v = zr[:].rearrange("p (g b one) -> p g b one", g=NCORES,
                                   one=1)
            nc.vector.reciprocal(zr_v, u[:, :, :, CH1:CH1Z])
            num, den = broadcast_tensor_aps(u[:, :, :, 0:CH1], zr_v)
            nc.vector.tensor_tensor(
                out=a_n[:].rearrange("p (g b k) -> p g b k", g=NCORES,
                                     b=BLOC),
                in0=num, in1=den, op=ALU.mult)
            at3.append(a_n)

        # yt row index is (b, o) = (g, w); M-halves split at g=4; each
        # half accumulates into one wide SBUF tile flushed by a single DMA
        for mh in range(2):
            c0 = mh * MH   # 0 or 96
            y_s = sc2.tile([MH, TLOC], F32, name=f"y_s{mh}_{r}",
                           tag=f"y_s{mh}", bufs=1)
            for m4 in range(EC):
                t0, t1 = m4 * 512, (m4 + 1) * 512
                yps = psC.tile([MH, 512], F32, name=f"yps{mh}{m4}_{r}",
                               tag="yps", bufs=3)
                for ec in range(EC):
                    nc.tensor.matmul(
                        yps[:], at3[ec][:, c0:c0 + MH],
                        wots[ec // 2][:, (ec % 2) * TLOC + t0:
                                      (ec % 2) * TLOC + t1],
                        start=(ec == 0), stop=False)
                nc.tensor.matmul(yps[:], w3sr_s[:, c0:c0 + MH],
                                 bot_s[:, t0:t1], start=False, stop=True)
                nc.vector.tensor_tensor(
                    out=y_s[:, t0:t1], in0=yps[:],
                    in1=xpall[:, mh * TLOC + t0:mh * TLOC + t1], op=ALU.add)
            nc.sync.dma_start(yt[c0:c0 + MH, :], y_s[:])


_CACHE = {}


def _get_program(reps: int, dbg: bool = False):
    key = (reps, dbg)
    if key not in _CACHE:
        _CACHE[key] = build_program(reps, dbg=dbg)
    return _CACHE[key]


class _PjrtRunner:
    """jit-once wrapper around bass2jax so repeat calls skip recompile/reload."""

    def __init__(self, nc):
        import jax
        from jax.sharding import Mesh, PartitionSpec
        from jax.experimental.shard_map import shard_map
        from concourse import bass2jax

        bass2jax.install_neuronx_cc_hook()
        self.nc = nc
        in_names, out_names, out_avals, zero_outs = [], [], [], []
        partition_name = (nc.partition_id_tensor.name
                          if nc.partition_id_tensor else None)
        for alloc in nc.m.functions[0].allocations:
            if not isinstance(alloc, mybir.MemoryLocationSet):
                continue
            name = alloc.memorylocations[0].name
            if alloc.kind == "ExternalInput":
                if name != partition_name:
                    in_names.append(name)
            elif alloc.kind == "ExternalOutput":
                shape = tuple(alloc.tensor_shape)
                dtype = mybir.dt.np(alloc.dtype)
                out_names.append(name)
                out_avals.append(jax.core.ShapedArray(shape, dtype))
                zero_outs.append(np.zeros(shape, dtype))
        self.n_params = len(in_names)
        self.in_names = list(in_names)
        self.out_names = out_names
        self.out_avals = out_avals
        self.zero_outs = zero_outs
        all_in_names = in_names + out_names
        if partition_name is not None:
            all_in_names.append(partition_name)

        n_outs = len(out_names)
        donate = tuple(range(self.n_params, self.n_params + n_outs))

        def _body(*args):
            operands = list(args)
            if partition_name is not None:
                operands.append(bass2jax.partition_id_tensor())
            outs = bass2jax._bass_exec_p.bind(
                *operands,
                out_avals=tuple(out_avals),
                in_names=tuple(all_in_names),
                out_names=tuple(out_names),
                lowering_input_output_aliases=(),
                sim_require_finite=True,
                sim_require_nnan=True,
                nc=nc,
            )
            return tuple(outs)

        devices = jax.devices()[:NCORES]
        mesh = Mesh(np.asarray(devices), ("core",))
        self.mesh = mesh
        in_specs = (PartitionSpec("core"),) * (self.n_params + n_outs)
        out_specs = (PartitionSpec("core"),) * n_outs
        self.fn = jax.jit(
            shard_map(_body, mesh=mesh, in_specs=in_specs,
                      out_specs=out_specs, check_rep=False),
            donate_argnums=donate, keep_unused=True)

    def __call__(self, in_maps):
        concat_in = [
            np.concatenate([np.asarray(in_maps[c][nm]) for c in range(NCORES)],
                           axis=0)
            for nm in self.in_names]
        concat_zeros = [
            np.zeros((NCORES * z.shape[0], *z.shape[1:]), z.dtype)
            for z in self.zero_outs]
        out_arrs = self.fn(*concat_in, *concat_zeros)
        return [
            {nm: np.asarray(out_arrs[i]).reshape(
                NCORES, *self.out_avals[i].shape)[c]
             for i, nm in enumerate(self.out_names)}
            for c in range(NCORES)]


_RUNNERS = {}


def _get_runner(reps: int, dbg: bool = False):
    key = (reps, dbg)
    if key not in _RUNNERS:
        _RUNNERS[key] = _PjrtRunner(_get_program(reps, dbg=dbg))
    return _RUNNERS[key]


def make_in_maps(x, W1, W2, Wq, bq, Wk, bk, Wv, bv, Wo, bo, W3):
    """Host-side sharding: slicing / transposition / constant assembly only."""
    f32, f16 = np.float32, np.float16
    x = np.asarray(x, f32)

    # Q mix: rows = the 11 reduced channels, cols = 16 Q outputs
    w12q = np.zeros((NCH, HID), f16)
    w12q[0:CH1, :] = np.asarray(W1, f32).T
    # K/(W3 V) mix: K -> cols 0..15; W3-folded V -> cols 32..34 (parked at
    # partition base 32 so its PE transpose is tile-position aligned);
    # col 35 = Z-ones row (filled by the bias plane)
    w3w2 = np.asarray(W3, f32) @ np.asarray(W2, f32)     # [3, 4]
    w12kv = np.zeros((NCH, NKV), f16)
    w12kv[CH1:CH1 + CH2, 0:HID] = np.asarray(W2, f32).T
    w12kv[CH1 + CH2:NCH, 2 * HID:2 * HID + CH1] = w3w2.T

    w3sum = np.asarray(W3, f32).sum(axis=1)              # [3]
    biasqkv = np.zeros((NKV, 2 * E), f32)
    biasqkv[0:HID, 0:E] = np.asarray(bq, f32)[None, :]
    biasqkv[0:HID, E:2 * E] = np.asarray(bk, f32)[None, :]
    biasqkv[2 * HID:2 * HID + CH1, E:2 * E] = (
        w3sum[:, None] * np.asarray(bv, f32)[None, :])
    biasqkv[NKV - 1, E:2 * E] = 1.0

    identt = np.zeros((NKV, CH1Z), f16)
    identt[0:CH1Z, :] = np.eye(CH1Z, dtype=f16)
    identt[2 * HID:NKV, :] = np.eye(CH1Z, dtype=f16)

    w3sr = np.tile(w3sum, B)[None, :].astype(f16)        # [1, 192], b*3+o

    in_maps = []
    for c in range(NCORES):
        sl = slice(c * TLOC, (c + 1) * TLOC)
        xt = np.ascontiguousarray(
            np.transpose(x[:, :, sl], (2, 1, 0)).reshape(TLOC, 7 * B))
        m = {
            "xt": xt.astype(f16),
            "xp": np.ascontiguousarray(
                x[:, :CH1, sl].reshape(CH1 * B, TLOC)).astype(f16),
            "wqkv": np.concatenate(
                [np.asarray(Wq, f32)[:, sl].T, np.asarray(Wk, f32)[:, sl].T,
                 np.asarray(Wv, f32)[:, sl].T], axis=1).astype(f16),
            "wot": np.asarray(Wo, f32)[sl, :].T.astype(f16),
            "bot": np.asarray(bo, f32)[sl][None, :].astype(f16),
            "w12q": w12q, "w12kv": w12kv, "biasqkv": biasqkv,
            "identt": identt, "w3sr": w3sr,
        }
        in_maps.append(m)
    return in_maps


def assemble_output(results):
    """[per-core yt [192, 2048]] -> [B, CH1, T]; row = b*CH1 + o."""
    arr = np.stack([res["yt"] for res in results], axis=0)  # [8, 192, 2048]
    return np.ascontiguousarray(
        arr.transpose(1, 0, 2).reshape(B, CH1, T))


def run(inputs, reps: int = 1, dbg: bool = False):
    runner = _get_runner(reps, dbg=dbg)
    in_maps = make_in_maps(**inputs)
    results = runner(in_maps)
    if dbg:
        return assemble_output(results), results
    return assemble_output(results)


def kernel(**inputs) -> np.ndarray:
    return run(inputs, reps=1)


def time_reps(inputs, reps: int, n: int = 10):
    """Per-call wall times with device-resident inputs (first call = warmup)."""
    import time
    import jax
    from jax.sharding import NamedSharding, PartitionSpec

    runner = _get_runner(reps)
    in_maps = make_in_maps(**inputs)
    concat = [
        np.concatenate([np.asarray(in_maps[c][nm]) for c in range(NCORES)],
                       axis=0)
        for nm in runner.in_names]
    sh = NamedSharding(runner.mesh, PartitionSpec("core"))
    dev = [jax.device_put(a, sh) for a in concat]
    times = []
    for i in range(n + 1):
        zeros = [np.zeros((NCORES * z.shape[0], *z.shape[1:]), z.dtype)
                 for z in runner.zero_outs]
        t0 = time.perf_counter()
        out = runner.fn(*dev, *zeros)
        jax.block_until_ready(out)
        dt = time.perf_counter() - t0
        if i > 0:
            times.append(dt)
    return times


# revision 5
# speedup vs baseline: 1.0057x; 1.0057x over previous
"""Trainium2 Bass kernel for nn_AttnResBlock (B=64, CH1=3, CH2=4, HID=16, T=16384, E=512).

Strategy: tensor-parallel split of the T dimension across 8 cores, fp16
datapath for every large tensor.

  y = p + W3 @ (attn(W1@p, W2@c | Wq,Wk,Wv) @ Wo)        p = x[:,:3], c = x[:,3:7]

The big weights (Wq/Wk/Wv [E,T], Wo [T,E]) dominate memory traffic.  Each
core owns a T-slice of 2048 and reads only its slice of each projection
weight; all large streams are fp16 (host-rounded), halving HBM bytes vs
fp32 while the PE accumulates in fp32 PSUM (matmul error ~1e-3 rel, well
inside the 2e-2 gate).

  stage A (T-parallel):   P[b,c,e]     = sum_{t in Ti} p[b,c,t] Wq[e,t]   (partial)
                          Ck/Cv[b,c,e] = sum_{t in Ti} c[b,c,t] W{k,v}[e,t]
    11 rows/batch (3 P + 4 Ck + 4 Cv) go through a fp16 ReduceScatter;
    core r receives batches 8r..8r+7.  Channel mixes and biases are applied
    AFTER the reduce (biases via a precomputed [49,E] bias plane added by
    DVE, so they ride along with the PSUM->SBUF eviction).
  stage B (B-parallel):   two block-diag matmuls produce Q [16,E] and K/V
                          [49,E] per batch (V parked at partition base 32 so
                          the V^T PE transposes are tile-position legal);
                          V^T via 4 tiny PE transposes (col 16 of each chunk
                          = 1.0 from the bias plane, which makes the attn
                          matmul also emit the softmax partition sums Z as
                          row 17); S^T = K^T Q, exp on ACT (fp16 out),
                          attn = V^T @ exp(S^T), W3-mix, 1/Z normalize.
                          Normalized attn3[e,(b,o)] is AllGather'ed
                          (tiny: 25KB/core).
  stage C (T-parallel):   y^T[(b,o), t] = at3^T Wo-slice contraction over e,
                          + bo*w3sum rank-1 term, + residual p (fp32 xp,
                          loaded during the collective window).

DMA instruction count is kept low (HWDGE dispatch costs ~625ns/DMA): weights
stream in 2-k-tile chunks, the ReduceScatter staging is 6 DMAs with
(ch,b)->(b,ch) row regrouping done by the DMA access pattern, and stage B
reads all of its post-scatter rows in a single DMA.
"""

import numpy as np

import concourse.bacc as bacc
import concourse.tile as tile
import concourse.mybir as mybir

F32 = mybir.dt.float32
F16 = mybir.dt.float16
AF = mybir.ActivationFunctionType
ALU = mybir.AluOpType

B, CH1, CH2, HID, T, E = 64, 3, 4, 16, 16384, 512
NCORES = 8
TLOC = T // NCORES          # 2048
KT = TLOC // 128            # 16 k-tiles in stage A
KCH = 4                     # k-tiles per DMA chunk
NCHUNK = KT // KCH          # 8 chunks
EC = E // 128               # 4 e/f chunks
BLOC = B // NCORES          # 8 batches per core in stage B
NCH = CH1 + CH2 + CH2       # 11 rows/batch through the ReduceScatter
NKV = 2 * HID + CH1 + 1     # 36 KV-mix rows: K 0..15, pad, W3V 32..34, Z 35
SCALE = 1.0 / np.sqrt(HID)  # attention scale
EXPSHIFT = 10.0             # exp(S*SCALE - 10): keeps exp inside fp16 range;
                            # softmax is shift-invariant so Z divides it out
CH1Z = CH1 + 1              # 3 W3-mixed channels + the softmax sum Z
GW = CH1Z * BLOC            # 32 AllGather columns per rank: (b, [o0 o1 o2 Z])


def build_program(reps: int = 1, dbg: bool = False):
    nc = bacc.Bacc("TRN2", target_bir_lowering=False, debug=False,
                   num_devices=NCORES)

    xt = nc.dram_tensor("xt", [TLOC, 7 * B], F16, kind="ExternalInput")
    xp = nc.dram_tensor("xp", [CH1 * B, TLOC], F16, kind="ExternalInput")
    wqkv = nc.dram_tensor("wqkv", [TLOC, 3 * E], F16, kind="ExternalInput")
    wot = nc.dram_tensor("wot", [E, TLOC], F16, kind="ExternalInput")
    bot = nc.dram_tensor("bot", [1, TLOC], F16, kind="ExternalInput")
    w12q = nc.dram_tensor("w12q", [NCH, HID], F16, kind="ExternalInput")
    w12kv = nc.dram_tensor("w12kv", [NCH, NKV], F16, kind="ExternalInput")
    biasqkv = nc.dram_tensor("biasqkv", [NKV, 2 * E], F32,
                             kind="ExternalInput")
    identt = nc.dram_tensor("identt", [NKV, CH1Z], F16,
                            kind="ExternalInput")
    w3sr = nc.dram_tensor("w3sr", [1, CH1 * B], F16, kind="ExternalInput")
    yt = nc.dram_tensor("yt", [CH1 * B, TLOC], F32, kind="ExternalOutput")

    rg = [list(range(NCORES))]

    with tile.TileContext(nc) as tc:
        with tc.tile_pool(name="const", bufs=1) as cp, \
             tc.tile_pool(name="dram", space="DRAM", bufs=1) as dp, \
             tc.tile_pool(name="xp", bufs=1) as xpool, \
             tc.tile_pool(name="wotp", bufs=1) as wotp:

            # small constants via the Pool (SWDGE) queue so they never
            # contend with the stage-A weight stream on HWDGE
            w12q_s = cp.tile([NCH, HID], F16, name="w12q_s")
            w12kv_s = cp.tile([NCH, NKV], F16, name="w12kv_s")
            biasqkv_s = cp.tile([NKV, 2 * E], F32, name="biasqkv_s")
            identt_s = cp.tile([NKV, CH1Z], F16, name="identt_s")
            w3sr_s = cp.tile([1, CH1 * B], F16, name="w3sr_s")
            bot_s = cp.tile([1, TLOC], F16, name="bot_s")
            nshift = cp.tile([128, 1], F32, name="nshift")
            nc.vector.memset(nshift[:], -EXPSHIFT)
            nc.gpsimd.dma_start(w12q_s[:], w12q[:])
            nc.gpsimd.dma_start(w12kv_s[:], w12kv[:])
            nc.gpsimd.dma_start(biasqkv_s[:], biasqkv[:])
            nc.gpsimd.dma_start(identt_s[:], identt[:])
            nc.gpsimd.dma_start(w3sr_s[:], w3sr[:])
            nc.gpsimd.dma_start(bot_s[:], bot[:])

            dbg_outs = {}
            if dbg:
                dbg_outs["dbg_rs"] = nc.dram_tensor(
                    "dbg_rs", [BLOC * NCH, E], F16, kind="ExternalOutput")
                dbg_outs["dbg_rsin"] = nc.dram_tensor(
                    "dbg_rsin", [B * NCH, E], F16, kind="ExternalOutput")
                dbg_outs["dbg_q"] = nc.dram_tensor(
                    "dbg_q", [HID, E], F16, kind="ExternalOutput")
                dbg_outs["dbg_kv"] = nc.dram_tensor(
                    "dbg_kv", [NKV, E], F16, kind="ExternalOutput")
                dbg_outs["dbg_et"] = nc.dram_tensor(
                    "dbg_et", [128, EC * E], F16, kind="ExternalOutput")
                dbg_outs["dbg_ag"] = nc.dram_tensor(
                    "dbg_ag", [NCORES * E, GW], F16, kind="ExternalOutput")
            for rep in range(reps):
                build_rep(nc, tc, dp, xpool, wotp, rep, rg, locals())

    nc.compile()
    return nc


def build_rep(nc, tc, dp, xpool, wotp, rep, rg, env):
    xt, xp, wqkv, wot, yt = (env[k] for k in
                             ("xt", "xp", "wqkv", "wot", "yt"))
    w12q_s, w12kv_s, biasqkv_s, identt_s = (
        env[k] for k in ("w12q_s", "w12kv_s", "biasqkv_s", "identt_s"))
    w3sr_s, bot_s, nshift = (env[k] for k in ("w3sr_s", "bot_s", "nshift"))
    dbg_outs = env.get("dbg_outs", {})

    r = f"r{rep}"

    # ---- DRAM bounce buffers for the collectives -------------------------
    rs_in = dp.tile([B * NCH, E], F16, name=f"rs_in_{r}", tag="rs_in", bufs=1)
    rs_out = dp.tile([BLOC * NCH, E], F16, name=f"rs_out_{r}", tag="rs_out",
                     bufs=1)
    ag_in = dp.tile([E, GW], F16, name=f"ag_in_{r}", tag="ag_in", bufs=1)
    ag_out = dp.tile([NCORES * E, GW], F16, name=f"ag_out_{r}", tag="ag_out",
                     bufs=1)

    # ---- stage A: big T-contraction ------------------------------------
    # x and the host-concatenated [wq|wk|wv] stream in k-tile chunks (two
    # HWDGE dispatches per chunk); small chunks first so the PE starts fast
    CHUNKS = (1, 1, 1, 1, 2, 2, 4, 4)
    assert sum(CHUNKS) == KT
    xts, wws = [], []
    k0c = 0
    for c, kch in enumerate(CHUNKS):
        xc = xpool.tile([128, kch * 7 * B], F16, name=f"x{c}_{r}",
                        tag=f"x{c}", bufs=1)
        wc = xpool.tile([128, kch * 3 * E], F16, name=f"ww{c}_{r}",
                        tag=f"ww{c}", bufs=1)
        rows = slice(k0c * 128, (k0c + kch) * 128)
        nc.sync.dma_start(
            xc[:].rearrange("p (a w) -> p a w", a=kch),
            xt[rows, :].rearrange("(a p) w -> p a w", a=kch))
        nc.sync.dma_start(
            wc[:].rearrange("p (a e) -> p a e", a=kch),
            wqkv[rows, :].rearrange("(a p) e -> p a e", a=kch))
        xts.append(xc)
        wws.append(wc)
        k0c += kch

    with tc.tile_pool(name="psA", space="PSUM", bufs=1) as psA, \
         tc.tile_pool(name="stgA", bufs=1) as sa:
        p0 = psA.tile([128, E], F32, name=f"p0_{r}", tag="p0", bufs=1)
        p1 = psA.tile([64, E], F32, name=f"p1_{r}", tag="p1", bufs=1)
        k0 = psA.tile([128, E], F32, name=f"k0_{r}", tag="k0", bufs=1)
        k1 = psA.tile([128, E], F32, name=f"k1_{r}", tag="k1", bufs=1)
        v0 = psA.tile([128, E], F32, name=f"v0_{r}", tag="v0", bufs=1)
        v1 = psA.tile([128, E], F32, name=f"v1_{r}", tag="v1", bufs=1)

        k0c = 0
        for c, kch in enumerate(CHUNKS):
            xc, wc = xts[c], wws[c]
            for a in range(kch):
                k = k0c + a
                st, sp = (k == 0), (k == KT - 1)
                x0 = a * 7 * B
                w0 = a * 3 * E
                wq_a = wc[:, w0:w0 + E]
                wk_a = wc[:, w0 + E:w0 + 2 * E]
                wv_a = wc[:, w0 + 2 * E:w0 + 3 * E]
                nc.tensor.matmul(p0[:], xc[:, x0:x0 + 128], wq_a,
                                 start=st, stop=sp)
                nc.tensor.matmul(p1[:], xc[:, x0 + 128:x0 + 192], wq_a,
                                 start=st, stop=sp)
                nc.tensor.matmul(k0[:], xc[:, x0 + 192:x0 + 320], wk_a,
                                 start=st, stop=sp)
                nc.tensor.matmul(v0[:], xc[:, x0 + 192:x0 + 320], wv_a,
                                 start=st, stop=sp)
                nc.tensor.matmul(k1[:], xc[:, x0 + 320:x0 + 448], wk_a,
                                 start=st, stop=sp)
                nc.tensor.matmul(v1[:], xc[:, x0 + 320:x0 + 448], wv_a,
                                 start=st, stop=sp)
            k0c += kch

        # PSUM -> SBUF fp16, split across the two PSUM-capable engines.
        # p0s/k0s/v0s live in the rep-long xpool, sized [128, 2*TLOC] (only
        # [:, :E] used) so the wot/xpall tiles below reuse their buffers:
        # the tag-rotation WAR dependency delays those big loads until the
        # staging DMAs have read the evictions — i.e. into the collective
        # window — instead of stealing stage-A DMA bandwidth.
        p0s = xpool.tile([128, 2 * TLOC], F16, name=f"p0s_{r}", tag="p0s",
                         bufs=1)
        p1s = sa.tile([64, E], F16, name=f"p1s_{r}", tag="p1s", bufs=1)
        k0s = xpool.tile([128, 2 * TLOC], F16, name=f"k0s_{r}", tag="k0s",
                         bufs=1)
        k1s = sa.tile([128, E], F16, name=f"k1s_{r}", tag="k1s", bufs=1)
        v0s = xpool.tile([128, 2 * TLOC], F16, name=f"v0s_{r}", tag="v0s",
                         bufs=1)
        v1s = sa.tile([128, E], F16, name=f"v1s_{r}", tag="v1s", bufs=1)
        nc.vector.tensor_copy(p0s[:, 0:E], p0[:])
        nc.scalar.activation(p1s[:], p1[:], AF.Copy)
        nc.vector.tensor_copy(k0s[:, 0:E], k0[:])
        nc.scalar.activation(k1s[:], k1[:], AF.Copy)
        nc.vector.tensor_copy(v0s[:, 0:E], v0[:])
        nc.scalar.activation(v1s[:], v1[:], AF.Copy)

        # (ch,b) rows -> rs_in's (b,ch) rows, regrouped by the DMA pattern.
        # ch layout per batch: [P0 P1 P2 | Ck0..3 | Cv0..3].  The SBUF side
        # stays a plain 2-d partition walk (same element order as the 3-d
        # DRAM view; dma_start only requires equal sizes).
        rs_v = rs_in[:].rearrange("(b c) e -> c b e", c=NCH)
        for s_t, ch0, nch in ((p0s, 0, 2), (p1s, 2, 1), (k0s, 3, 2),
                              (k1s, 5, 2), (v0s, 7, 2), (v1s, 9, 2)):
            nc.sync.dma_start(rs_v[ch0:ch0 + nch], s_t[:, 0:E])

    nc.gpsimd.collective_compute(
        "ReduceScatter", ALU.add, replica_groups=rg,
        ins=[rs_in.opt()], outs=[rs_out.opt()],
    )
    if rep == 0 and "dbg_rs" in dbg_outs:
        nc.gpsimd.dma_start(dbg_outs["dbg_rs"][:], rs_out[:])
        nc.gpsimd.dma_start(dbg_outs["dbg_rsin"][:], rs_in[:])

    # all post-scatter rows in one DMA, channel-major so every per-batch
    # slice starts at partition 0 (engines cannot shift partition lanes);
    # emitted BEFORE the wot/xpall loads so it dispatches the moment the
    # ReduceScatter completes instead of queueing behind them
    rs_all = xpool.tile([NCH, BLOC * E], F16, name=f"rs_all_{r}",
                        tag="rs_all", bufs=1)
    nc.sync.dma_start(
        rs_all[:].rearrange("c (b e) -> c b e", b=BLOC),
        rs_out[:].rearrange("(b c) e -> c b e", c=NCH))

    # stage-C inputs, buffer-reusing the eviction tags (see above): the
    # WAR dependency releases them at the stage-A tail, so they stream
    # during the collective window
    wots = []
    for half, tag in ((0, "p0s"), (1, "k0s")):
        wo_t = xpool.tile([128, 2 * TLOC], F16, name=f"wo{half}_{r}",
                          tag=tag, bufs=1)
        nc.sync.dma_start(
            wo_t[:].rearrange("p (a t) -> p a t", a=2),
            wot[half * 256:(half + 1) * 256, :].rearrange(
                "(a p) t -> p a t", a=2))
        wots.append(wo_t)
    xpbig = xpool.tile([128, 2 * TLOC], F16, name=f"xpall_{r}", tag="v0s",
                       bufs=1)
    xpall = xpbig[0:96, :]
    nc.sync.dma_start(
        xpall.rearrange("p (m t) -> p m t", m=2),
        xp[:].rearrange("(m p) t -> p m t", m=2))

    # ---- stage B: per-batch attention ----------------------------------
    with tc.tile_pool(name="psB", space="PSUM", bufs=1) as psB, \
         tc.tile_pool(name="sbB", bufs=2) as sb:
        ag_s = sb.tile([128, EC * GW], F16, name=f"ag_s_{r}", tag="ag_s",
                       bufs=1)

        # per-batch state carried across the 1-batch software-pipeline skew
        qkv_sL = [None] * BLOC
        vt_sL = [None] * BLOC
        etL = [None] * BLOC
        apsL = [None] * BLOC

        def emit_mix(b):
            """channel mixes for batch b: Q and K/W3V/Z in separate PSUM
            tiles so the two DVE evictions (which also apply the biases)
            free their banks independently and pipeline with the consumers.
            qkv_s layout: columns 0..E-1 = Q (rows 0..15), E..2E-1 = K
            (rows 0..15) / W3V+ones (rows 32..35)."""
            rs_b = rs_all[:, b * E:(b + 1) * E]
            q_ps = psB.tile([HID, E], F32, name=f"qps{b}_{r}", tag="qps",
                            bufs=1)
            kv_ps = psB.tile([NKV, E], F32, name=f"kvps{b}_{r}", tag="kvps",
                             bufs=1)
            nc.tensor.matmul(q_ps[:], w12q_s[:], rs_b, start=True, stop=True)
            nc.tensor.matmul(kv_ps[:], w12kv_s[:], rs_b, start=True,
                             stop=True)
            qkv_s = sb.tile([NKV, 2 * E], F16, name=f"qkv_s{b}_{r}",
                            tag="qkv_s", bufs=2)
            nc.vector.tensor_tensor(out=qkv_s[0:HID, 0:E], in0=q_ps[:],
                                    in1=biasqkv_s[0:HID, 0:E], op=ALU.add)
            nc.vector.tensor_tensor(out=qkv_s[:, E:2 * E], in0=kv_ps[:],
                                    in1=biasqkv_s[:, E:2 * E], op=ALU.add)
            qkv_sL[b] = qkv_s

        def emit_vt(b):
            """(W3V | 1)^T: rows 32..35 of the K-half -> [128, 4] per
            f-chunk.  vtp and atp share PSUM banks (tag "tp")."""
            qkv_s = qkv_sL[b]
            vtp = psB.tile([128, EC * CH1Z], F16, name=f"vtp{b}_{r}",
                           tag="tp", bufs=1)
            for fc in range(EC):
                nc.tensor.transpose(
                    vtp[:, fc * CH1Z:(fc + 1) * CH1Z],
                    qkv_s[2 * HID:NKV, E + fc * 128:E + (fc + 1) * 128],
                    identt_s[2 * HID:NKV, :])
            vt_s = sb.tile([128, EC * CH1Z], F16, name=f"vt_s{b}_{r}",
                           tag="vt_s", bufs=2)
            nc.vector.tensor_copy(vt_s[:], vtp[:])
            vt_sL[b] = vt_s

        def emit_s(b):
            """four S^T chunk matmuls, one exp each (chunk granularity
            keeps the PE->ACT pipeline fine-grained)."""
            qkv_s = qkv_sL[b]
            q_ap = qkv_s[0:HID, 0:E]
            ets = []
            for fc in range(EC):
                sps = psB.tile([128, E], F32, name=f"sps{b}{fc}_{r}",
                               tag="s", bufs=2)
                nc.tensor.matmul(
                    sps[:], qkv_s[0:HID, E + fc * 128:E + (fc + 1) * 128],
                    q_ap, start=True, stop=True)
                et = sb.tile([128, E], F16, name=f"et{b}{fc}_{r}", tag="et",
                             bufs=8)
                nc.scalar.activation(et[:], sps[:], AF.Exp, scale=SCALE,
                                     bias=nshift[:])
                ets.append(et)
            etL[b] = ets

        def emit_attn(b):
            """attn matmuls for batch b (needs et[b] ready).

            Rows: 0..2 = W3-mixed attention (unnormalized), 3 = Z."""
            aps = psB.tile([CH1Z, E], F32, name=f"aps{b}_{r}", tag="attn",
                           bufs=2)
            vt_s, ets = vt_sL[b], etL[b]
            for fc in range(EC):
                nc.tensor.matmul(aps[:], vt_s[:, fc * CH1Z:(fc + 1) * CH1Z],
                                 ets[fc][:], start=(fc == 0), stop=(fc == 3))
            apsL[b] = aps

        def emit_back(b):
            """transpose attn rows into e-partition layout for the gather."""
            aps = apsL[b]
            an_s = sb.tile([CH1Z, E], F16, name=f"an_s{b}_{r}", tag="an_s",
                           bufs=2)
            nc.vector.tensor_copy(an_s[:], aps[:])
            atp = psB.tile([128, EC * CH1Z], F16, name=f"atp{b}_{r}",
                           tag="atp", bufs=1)
            for ec in range(EC):
                nc.tensor.transpose(
                    atp[:, ec * CH1Z:(ec + 1) * CH1Z],
                    an_s[:, ec * 128:(ec + 1) * 128],
                    identt_s[0:CH1Z, :])
            # ag_s column layout per e-chunk block: (b, [o0 o1 o2 Z])
            nc.vector.tensor_copy(
                ag_s[:].rearrange("p (c q) -> p c q", c=EC)[
                    :, :, b * CH1Z:(b + 1) * CH1Z],
                atp[:].rearrange("p (c k) -> p c k", c=EC))

        # software pipeline with a 1-batch skew: after each mix, the PE
        # chews on batch b-1 (attn + output transposes) while the DVE
        # evicts batch b's mixes, so no engine waits on the
        # mix->evict->S chain
        emit_mix(0)
        emit_vt(0)
        emit_s(0)
        for b in range(1, BLOC):
            emit_mix(b)
            emit_attn(b - 1)
            emit_back(b - 1)
            emit_vt(b)
            emit_s(b)
        emit_attn(BLOC - 1)
        emit_back(BLOC - 1)

        if rep == 0 and "dbg_q" in dbg_outs:
            nc.gpsimd.dma_start(dbg_outs["dbg_q"][:], qkv_sL[0][0:HID, 0:E])
            nc.gpsimd.dma_start(dbg_outs["dbg_kv"][:],
                                qkv_sL[0][:, E:2 * E])
        if rep == 0 and "dbg_et" in dbg_outs:
            nc.gpsimd.dma_start(dbg_outs["dbg_et"][:], etL[0][:])

        nc.sync.dma_start(
            ag_in[:].rearrange("(c p) w -> p c w", c=EC),
            ag_s[:].rearrange("p (c w) -> p c w", c=EC))

    nc.gpsimd.collective_compute(
        "AllGather", ALU.bypass, replica_groups=rg,
        ins=[ag_in.opt()], outs=[ag_out.opt()],
    )
    if rep == 0 and "dbg_ag" in dbg_outs:
        nc.gpsimd.dma_start(dbg_outs["dbg_ag"][:], ag_out[:])

    # ---- stage C: y^T[(b,o), t] = at3^T Wo^T + bias + residual ----------
    from concourse.bass import broadcast_tensor_aps

    with tc.tile_pool(name="psC", space="PSUM", bufs=1) as psC, \
         tc.tile_pool(name="sbC", bufs=1) as sc2:
        MH = CH1 * B // 2      # 96 (b,o) rows per M-half
        atall = sc2.tile([128, EC * NCORES * GW], F16, name=f"atall_{r}",
                         tag="atall", bufs=1)
        ag_v = ag_out[:].rearrange("(g c p) w -> c p g w", g=NCORES, c=EC)
        for ec in range(EC):
            nc.gpsimd.dma_start(
                atall[:, ec * NCORES * GW:(ec + 1) * NCORES * GW].rearrange(
                    "p (g w) -> p g w", g=NCORES),
                ag_v[ec])
        at3 = []
        for ec in range(EC):
            a_u = atall[:, ec * NCORES * GW:(ec + 1) * NCORES * GW]
            # normalize: at3[e, (g,b,o)] = att[e,(g,b,o)] * (1/Z[e,(g,b)])
            a_n = sc2.tile([128, CH1 * B], F16, name=f"at3{ec}_{r}",
                           tag=f"at3{ec}", bufs=1)
            u = a_u.rearrange("p (g b k) -> p g b k", g=NCORES, b=BLOC)
            zr = sc2.tile([128, B], F32, name=f"zr{ec}_{r}", tag=f"zr{ec}",
                          bufs=1)
            zr_v = zr[:].rearrange("p (g b one) -> p g b one", g=NCORES,
                                   one=1)
            nc.vector.reciprocal(zr_v, u[:, :, :, CH1:CH1Z])
            num, den = broadcast_tensor_aps(u[:, :, :, 0:CH1], zr_v)
            nc.vector.tensor_tensor(
                out=a_n[:].rearrange("p (g b k) -> p g b k", g=NCORES,
                                     b=BLOC),
                in0=num, in1=den, op=ALU.mult)
            at3.append(a_n)

        # yt row index is (b, o) = (g, w); M-halves split at g=4; each
        # half accumulates into one wide SBUF tile flushed by a single DMA
        for mh in range(2):
            c0 = mh * MH   # 0 or 96
            y_s = sc2.tile([MH, TLOC], F32, name=f"y_s{mh}_{r}",
                           tag=f"y_s{mh}", bufs=1)
            for m4 in range(EC):
                t0, t1 = m4 * 512, (m4 + 1) * 512
                yps = psC.tile([MH, 512], F32, name=f"yps{mh}{m4}_{r}",
                               tag="yps", bufs=3)
                # bias rank-1 first: it only needs constants, so it runs
                # during the AllGather window and keeps the PE warm
                nc.tensor.matmul(yps[:], w3sr_s[:, c0:c0 + MH],
                                 bot_s[:, t0:t1], start=True, stop=False)
                for ec in range(EC):
                    nc.tensor.matmul(
                        yps[:], at3[ec][:, c0:c0 + MH],
                        wots[ec // 2][:, (ec % 2) * TLOC + t0:
                                      (ec % 2) * TLOC + t1],
                        start=False, stop=(ec == EC - 1))
                nc.vector.tensor_tensor(
                    out=y_s[:, t0:t1], in0=yps[:],
                    in1=xpall[:, mh * TLOC + t0:mh * TLOC + t1], op=ALU.add)
            nc.sync.dma_start(yt[c0:c0 + MH, :], y_s[:])


_CACHE = {}


def _get_program(reps: int, dbg: bool = False):
    key = (reps, dbg)
    if key not in _CACHE:
        _CACHE[key] = build_program(reps, dbg=dbg)
    return _CACHE[key]


class _PjrtRunner:
    """jit-once wrapper around bass2jax so repeat calls skip recompile/reload."""

    def __init__(self, nc):
        import jax
        from jax.sharding import Mesh, PartitionSpec
        from jax.experimental.shard_map import shard_map
        from concourse import bass2jax

        bass2jax.install_neuronx_cc_hook()
        self.nc = nc
        in_names, out_names, out_avals, zero_outs = [], [], [], []
        partition_name = (nc.partition_id_tensor.name
                          if nc.partition_id_tensor else None)
        for alloc in nc.m.functions[0].allocations:
            if not isinstance(alloc, mybir.MemoryLocationSet):
                continue
            name = alloc.memorylocations[0].name
            if alloc.kind == "ExternalInput":
                if name != partition_name:
                    in_names.append(name)
            elif alloc.kind == "ExternalOutput":
                shape = tuple(alloc.tensor_shape)
                dtype = mybir.dt.np(alloc.dtype)
                out_names.append(name)
                out_avals.append(jax.core.ShapedArray(shape, dtype))
                zero_outs.append(np.zeros(shape, dtype))
        self.n_params = len(in_names)
        self.in_names = list(in_names)
        self.out_names = out_names
        self.out_avals = out_avals
        self.zero_outs = zero_outs
        all_in_names = in_names + out_names
        if partition_name is not None:
            all_in_names.append(partition_name)

        n_outs = len(out_names)
        donate = tuple(range(self.n_params, self.n_params + n_outs))

        def _body(*args):
            operands = list(args)
            if partition_name is not None:
                operands.append(bass2jax.partition_id_tensor())
            outs = bass2jax._bass_exec_p.bind(
                *operands,
                out_avals=tuple(out_avals),
                in_names=tuple(all_in_names),
                out_names=tuple(out_names),
                lowering_input_output_aliases=(),
                sim_require_finite=True,
                sim_require_nnan=True,
                nc=nc,
            )
            return tuple(outs)

        devices = jax.devices()[:NCORES]
        mesh = Mesh(np.asarray(devices), ("core",))
        self.mesh = mesh
        in_specs = (PartitionSpec("core"),) * (self.n_params + n_outs)
        out_specs = (PartitionSpec("core"),) * n_outs
        self.fn = jax.jit(
            shard_map(_body, mesh=mesh, in_specs=in_specs,
                      out_specs=out_specs, check_rep=False),
            donate_argnums=donate, keep_unused=True)

    def __call__(self, in_maps):
        concat_in = [
            np.concatenate([np.asarray(in_maps[c][nm]) for c in range(NCORES)],
                           axis=0)
            for nm in self.in_names]
        concat_zeros = [
            np.zeros((NCORES * z.shape[0], *z.shape[1:]), z.dtype)
            for z in self.zero_outs]
        out_arrs = self.fn(*concat_in, *concat_zeros)
        return [
            {nm: np.asarray(out_arrs[i]).reshape(
                NCORES, *self.out_avals[i].shape)[c]
             for i, nm in enumerate(self.out_names)}
            for c in range(NCORES)]


_RUNNERS = {}


def _get_runner(reps: int, dbg: bool = False):
    key = (reps, dbg)
    if key not in _RUNNERS:
        _RUNNERS[key] = _PjrtRunner(_get_program(reps, dbg=dbg))
    return _RUNNERS[key]


def make_in_maps(x, W1, W2, Wq, bq, Wk, bk, Wv, bv, Wo, bo, W3):
    """Host-side sharding: slicing / transposition / constant assembly only."""
    f32, f16 = np.float32, np.float16
    x = np.asarray(x, f32)

    # Q mix: rows = the 11 reduced channels, cols = 16 Q outputs
    w12q = np.zeros((NCH, HID), f16)
    w12q[0:CH1, :] = np.asarray(W1, f32).T
    # K/(W3 V) mix: K -> cols 0..15; W3-folded V -> cols 32..34 (parked at
    # partition base 32 so its PE transpose is tile-position aligned);
    # col 35 = Z-ones row (filled by the bias plane)
    w3w2 = np.asarray(W3, f32) @ np.asarray(W2, f32)     # [3, 4]
    w12kv = np.zeros((NCH, NKV), f16)
    w12kv[CH1:CH1 + CH2, 0:HID] = np.asarray(W2, f32).T
    w12kv[CH1 + CH2:NCH, 2 * HID:2 * HID + CH1] = w3w2.T

    w3sum = np.asarray(W3, f32).sum(axis=1)              # [3]
    biasqkv = np.zeros((NKV, 2 * E), f32)
    biasqkv[0:HID, 0:E] = np.asarray(bq, f32)[None, :]
    biasqkv[0:HID, E:2 * E] = np.asarray(bk, f32)[None, :]
    biasqkv[2 * HID:2 * HID + CH1, E:2 * E] = (
        w3sum[:, None] * np.asarray(bv, f32)[None, :])
    biasqkv[NKV - 1, E:2 * E] = 1.0

    identt = np.zeros((NKV, CH1Z), f16)
    identt[0:CH1Z, :] = np.eye(CH1Z, dtype=f16)
    identt[2 * HID:NKV, :] = np.eye(CH1Z, dtype=f16)

    w3sr = np.tile(w3sum, B)[None, :].astype(f16)        # [1, 192], b*3+o

    in_maps = []
    for c in range(NCORES):
        sl = slice(c * TLOC, (c + 1) * TLOC)
        xt = np.ascontiguousarray(
            np.transpose(x[:, :, sl], (2, 1, 0)).reshape(TLOC, 7 * B))
        m = {
            "xt": xt.astype(f16),
            "xp": np.ascontiguousarray(
                x[:, :CH1, sl].reshape(CH1 * B, TLOC)).astype(f16),
            "wqkv": np.concatenate(
                [np.asarray(Wq, f32)[:, sl].T, np.asarray(Wk, f32)[:, sl].T,
                 np.asarray(Wv, f32)[:, sl].T], axis=1).astype(f16),
            "wot": np.asarray(Wo, f32)[sl, :].T.astype(f16),
            "bot": np.asarray(bo, f32)[sl][None, :].astype(f16),
            "w12q": w12q, "w12kv": w12kv, "biasqkv": biasqkv,
            "identt": identt, "w3sr": w3sr,
        }
        in_maps.append(m)
    return in_maps


def assemble_output(results):
    """[per-core yt [192, 2048]] -> [B, CH1, T]; row = b*CH1 + o."""
    arr = np.stack([res["yt"] for res in results], axis=0)  # [8, 192, 2048]
    return np.ascontiguousarray(
        arr.transpose(1, 0, 2).reshape(B, CH1, T))


def run(inputs, reps: int = 1, dbg: bool = False):
    runner = _get_runner(reps, dbg=dbg)
    in_maps = make_in_maps(**inputs)
    results = runner(in_maps)
    if dbg:
        return assemble_output(results), results
    return assemble_output(results)


def kernel(**inputs) -> np.ndarray:
    return run(inputs, reps=1)


def time_reps(inputs, reps: int, n: int = 10):
    """Per-call wall times with device-resident inputs (first call = warmup)."""
    import time
    import jax
    from jax.sharding import NamedSharding, PartitionSpec

    runner = _get_runner(reps)
    in_maps = make_in_maps(**inputs)
    concat = [
        np.concatenate([np.asarray(in_maps[c][nm]) for c in range(NCORES)],
                       axis=0)
        for nm in runner.in_names]
    sh = NamedSharding(runner.mesh, PartitionSpec("core"))
    dev = [jax.device_put(a, sh) for a in concat]
    times = []
    for i in range(n + 1):
        zeros = [np.zeros((NCORES * z.shape[0], *z.shape[1:]), z.dtype)
                 for z in runner.zero_outs]
        t0 = time.perf_counter()
        out = runner.fn(*dev, *zeros)
        jax.block_until_ready(out)
        dt = time.perf_counter() - t0
        if i > 0:
            times.append(dt)
    return times


# revision 7
# speedup vs baseline: 1.0302x; 1.0244x over previous
"""Trainium2 Bass kernel for nn_AttnResBlock (B=64, CH1=3, CH2=4, HID=16, T=16384, E=512).

Strategy: tensor-parallel split of the T dimension across 8 cores, fp16
datapath for every large tensor.

  y = p + W3 @ (attn(W1@p, W2@c | Wq,Wk,Wv) @ Wo)        p = x[:,:3], c = x[:,3:7]

The big weights (Wq/Wk/Wv [E,T], Wo [T,E]) dominate memory traffic.  Each
core owns a T-slice of 2048 and reads only its slice of each projection
weight; all large streams are fp16 (host-rounded), halving HBM bytes vs
fp32 while the PE accumulates in fp32 PSUM (matmul error ~1e-3 rel, well
inside the 2e-2 gate).

  stage A (T-parallel):   P[b,c,e]     = sum_{t in Ti} p[b,c,t] Wq[e,t]   (partial)
                          Ck/Cv[b,c,e] = sum_{t in Ti} c[b,c,t] W{k,v}[e,t]
    11 rows/batch (3 P + 4 Ck + 4 Cv) go through a fp16 ReduceScatter;
    core r receives batches 8r..8r+7.  Channel mixes and biases are applied
    AFTER the reduce (biases via a precomputed [49,E] bias plane added by
    DVE, so they ride along with the PSUM->SBUF eviction).
  stage B (B-parallel):   two block-diag matmuls produce Q [16,E] and K/V
                          [49,E] per batch (V parked at partition base 32 so
                          the V^T PE transposes are tile-position legal);
                          V^T via 4 tiny PE transposes (col 16 of each chunk
                          = 1.0 from the bias plane, which makes the attn
                          matmul also emit the softmax partition sums Z as
                          row 17); S^T = K^T Q, exp on ACT (fp16 out),
                          attn = V^T @ exp(S^T), W3-mix, 1/Z normalize.
                          Normalized attn3[e,(b,o)] is AllGather'ed
                          (tiny: 25KB/core).
  stage C (T-parallel):   y^T[(b,o), t] = at3^T Wo-slice contraction over e,
                          + bo*w3sum rank-1 term, + residual p (fp32 xp,
                          loaded during the collective window).

DMA instruction count is kept low (HWDGE dispatch costs ~625ns/DMA): weights
stream in 2-k-tile chunks, the ReduceScatter staging is 6 DMAs with
(ch,b)->(b,ch) row regrouping done by the DMA access pattern, and stage B
reads all of its post-scatter rows in a single DMA.
"""

import numpy as np

import concourse.bacc as bacc
import concourse.tile as tile
import concourse.mybir as mybir

F32 = mybir.dt.float32
F16 = mybir.dt.float16
AF = mybir.ActivationFunctionType
ALU = mybir.AluOpType

B, CH1, CH2, HID, T, E = 64, 3, 4, 16, 16384, 512
NCORES = 8
TLOC = T // NCORES          # 2048
KT = TLOC // 128            # 16 k-tiles in stage A
KCH = 4                     # k-tiles per DMA chunk
NCHUNK = KT // KCH          # 8 chunks
EC = E // 128               # 4 e/f chunks
BLOC = B // NCORES          # 8 batches per core in stage B
NCH = CH1 + CH2 + CH2       # 11 rows/batch through the ReduceScatter
NKV = 2 * HID + CH1 + 1     # 36 KV-mix rows: K 0..15, pad, W3V 32..34, Z 35
SCALE = 1.0 / np.sqrt(HID)  # attention scale
EXPSHIFT = 10.0             # exp(S*SCALE - 10): keeps exp inside fp16 range;
                            # softmax is shift-invariant so Z divides it out
CH1Z = CH1 + 1              # 3 W3-mixed channels + the softmax sum Z
GW = CH1Z * BLOC            # 32 AllGather columns per rank: (b, [o0 o1 o2 Z])


def build_program(reps: int = 1, dbg: bool = False):
    nc = bacc.Bacc("TRN2", target_bir_lowering=False, debug=False,
                   num_devices=NCORES)

    xt = nc.dram_tensor("xt", [TLOC, 7 * B], F16, kind="ExternalInput")
    xp = nc.dram_tensor("xp", [CH1 * B, TLOC], F16, kind="ExternalInput")
    wqkv = nc.dram_tensor("wqkv", [TLOC, 3 * E], F16, kind="ExternalInput")
    wot = nc.dram_tensor("wot", [E, TLOC], F16, kind="ExternalInput")
    bot = nc.dram_tensor("bot", [1, TLOC], F16, kind="ExternalInput")
    w12q = nc.dram_tensor("w12q", [NCH, HID], F16, kind="ExternalInput")
    w12kv = nc.dram_tensor("w12kv", [NCH, NKV], F16, kind="ExternalInput")
    biasqkv = nc.dram_tensor("biasqkv", [NKV, 2 * E], F32,
                             kind="ExternalInput")
    identt = nc.dram_tensor("identt", [NKV, CH1Z], F16,
                            kind="ExternalInput")
    w3sr = nc.dram_tensor("w3sr", [1, CH1 * B], F16, kind="ExternalInput")
    yt = nc.dram_tensor("yt", [CH1 * B, TLOC], F32, kind="ExternalOutput")

    rg = [list(range(NCORES))]

    with tile.TileContext(nc) as tc:
        with tc.tile_pool(name="const", bufs=1) as cp, \
             tc.tile_pool(name="dram", space="DRAM", bufs=1) as dp, \
             tc.tile_pool(name="xp", bufs=1) as xpool, \
             tc.tile_pool(name="wotp", bufs=1) as wotp:

            # small constants via the Pool (SWDGE) queue so they never
            # contend with the stage-A weight stream on HWDGE
            w12q_s = cp.tile([NCH, HID], F16, name="w12q_s")
            w12kv_s = cp.tile([NCH, NKV], F16, name="w12kv_s")
            biasqkv_s = cp.tile([NKV, 2 * E], F32, name="biasqkv_s")
            identt_s = cp.tile([NKV, CH1Z], F16, name="identt_s")
            w3sr_s = cp.tile([1, CH1 * B], F16, name="w3sr_s")
            bot_s = cp.tile([1, TLOC], F16, name="bot_s")
            nshift = cp.tile([128, 1], F32, name="nshift")
            nc.vector.memset(nshift[:], -EXPSHIFT)
            nc.gpsimd.dma_start(w12q_s[:], w12q[:])
            nc.gpsimd.dma_start(w12kv_s[:], w12kv[:])
            nc.gpsimd.dma_start(biasqkv_s[:], biasqkv[:])
            nc.gpsimd.dma_start(identt_s[:], identt[:])
            nc.gpsimd.dma_start(w3sr_s[:], w3sr[:])
            nc.gpsimd.dma_start(bot_s[:], bot[:])

            dbg_outs = {}
            if dbg:
                dbg_outs["dbg_rs"] = nc.dram_tensor(
                    "dbg_rs", [BLOC * NCH, E], F16, kind="ExternalOutput")
                dbg_outs["dbg_rsin"] = nc.dram_tensor(
                    "dbg_rsin", [B * NCH, E], F16, kind="ExternalOutput")
                dbg_outs["dbg_q"] = nc.dram_tensor(
                    "dbg_q", [HID, E], F16, kind="ExternalOutput")
                dbg_outs["dbg_kv"] = nc.dram_tensor(
                    "dbg_kv", [NKV, E], F16, kind="ExternalOutput")
                dbg_outs["dbg_et"] = nc.dram_tensor(
                    "dbg_et", [128, EC * E], F16, kind="ExternalOutput")
                dbg_outs["dbg_ag"] = nc.dram_tensor(
                    "dbg_ag", [NCORES * E, GW], F16, kind="ExternalOutput")
            for rep in range(reps):
                build_rep(nc, tc, dp, xpool, wotp, rep, rg, locals())

    nc.compile()
    return nc


def build_rep(nc, tc, dp, xpool, wotp, rep, rg, env):
    xt, xp, wqkv, wot, yt = (env[k] for k in
                             ("xt", "xp", "wqkv", "wot", "yt"))
    w12q_s, w12kv_s, biasqkv_s, identt_s = (
        env[k] for k in ("w12q_s", "w12kv_s", "biasqkv_s", "identt_s"))
    w3sr_s, bot_s, nshift = (env[k] for k in ("w3sr_s", "bot_s", "nshift"))
    dbg_outs = env.get("dbg_outs", {})

    r = f"r{rep}"

    # ---- DRAM bounce buffers for the collectives -------------------------
    rs_in = dp.tile([B * NCH, E], F16, name=f"rs_in_{r}", tag="rs_in", bufs=1)
    rs_out = dp.tile([BLOC * NCH, E], F16, name=f"rs_out_{r}", tag="rs_out",
                     bufs=1)
    ag_in = dp.tile([E, GW], F16, name=f"ag_in_{r}", tag="ag_in", bufs=1)
    ag_out = dp.tile([NCORES * E, GW], F16, name=f"ag_out_{r}", tag="ag_out",
                     bufs=1)

    # ---- stage A: big T-contraction ------------------------------------
    # x and the host-concatenated [wq|wk|wv] stream in k-tile chunks (two
    # HWDGE dispatches per chunk); small chunks first so the PE starts fast
    CHUNKS = (1, 1, 1, 1, 2, 2, 4, 4)
    assert sum(CHUNKS) == KT
    xts, wws = [], []
    k0c = 0
    for c, kch in enumerate(CHUNKS):
        xc = xpool.tile([128, kch * 7 * B], F16, name=f"x{c}_{r}",
                        tag=f"x{c}", bufs=1)
        wc = xpool.tile([128, kch * 3 * E], F16, name=f"ww{c}_{r}",
                        tag=f"ww{c}", bufs=1)
        rows = slice(k0c * 128, (k0c + kch) * 128)
        nc.sync.dma_start(
            xc[:].rearrange("p (a w) -> p a w", a=kch),
            xt[rows, :].rearrange("(a p) w -> p a w", a=kch))
        nc.sync.dma_start(
            wc[:].rearrange("p (a e) -> p a e", a=kch),
            wqkv[rows, :].rearrange("(a p) e -> p a e", a=kch))
        xts.append(xc)
        wws.append(wc)
        k0c += kch

    with tc.tile_pool(name="psA", space="PSUM", bufs=1) as psA, \
         tc.tile_pool(name="stgA", bufs=1) as sa:
        p0 = psA.tile([128, E], F32, name=f"p0_{r}", tag="p0", bufs=1)
        p1 = psA.tile([64, E], F32, name=f"p1_{r}", tag="p1", bufs=1)
        k0 = psA.tile([128, E], F32, name=f"k0_{r}", tag="k0", bufs=1)
        k1 = psA.tile([128, E], F32, name=f"k1_{r}", tag="k1", bufs=1)
        v0 = psA.tile([128, E], F32, name=f"v0_{r}", tag="v0", bufs=1)
        v1 = psA.tile([128, E], F32, name=f"v1_{r}", tag="v1", bufs=1)

        k0c = 0
        for c, kch in enumerate(CHUNKS):
            xc, wc = xts[c], wws[c]
            for a in range(kch):
                k = k0c + a
                st, sp = (k == 0), (k == KT - 1)
                x0 = a * 7 * B
                w0 = a * 3 * E
                wq_a = wc[:, w0:w0 + E]
                wk_a = wc[:, w0 + E:w0 + 2 * E]
                wv_a = wc[:, w0 + 2 * E:w0 + 3 * E]
                nc.tensor.matmul(p0[:], xc[:, x0:x0 + 128], wq_a,
                                 start=st, stop=sp)
                nc.tensor.matmul(p1[:], xc[:, x0 + 128:x0 + 192], wq_a,
                                 start=st, stop=sp)
                nc.tensor.matmul(k0[:], xc[:, x0 + 192:x0 + 320], wk_a,
                                 start=st, stop=sp)
                nc.tensor.matmul(v0[:], xc[:, x0 + 192:x0 + 320], wv_a,
                                 start=st, stop=sp)
                nc.tensor.matmul(k1[:], xc[:, x0 + 320:x0 + 448], wk_a,
                                 start=st, stop=sp)
                nc.tensor.matmul(v1[:], xc[:, x0 + 320:x0 + 448], wv_a,
                                 start=st, stop=sp)
            k0c += kch

        # PSUM -> SBUF fp16, split across the two PSUM-capable engines.
        # p0s/k0s/v0s live in the rep-long xpool, sized [128, 2*TLOC] (only
        # [:, :E] used) so the wot/xpall tiles below reuse their buffers:
        # the tag-rotation WAR dependency delays those big loads until the
        # staging DMAs have read the evictions — i.e. into the collective
        # window — instead of stealing stage-A DMA bandwidth.
        p0s = xpool.tile([128, 2 * TLOC], F16, name=f"p0s_{r}", tag="p0s",
                         bufs=1)
        p1s = sa.tile([64, E], F16, name=f"p1s_{r}", tag="p1s", bufs=1)
        k0s = xpool.tile([128, 2 * TLOC], F16, name=f"k0s_{r}", tag="k0s",
                         bufs=1)
        k1s = sa.tile([128, E], F16, name=f"k1s_{r}", tag="k1s", bufs=1)
        v0s = xpool.tile([128, 2 * TLOC], F16, name=f"v0s_{r}", tag="v0s",
                         bufs=1)
        v1s = sa.tile([128, E], F16, name=f"v1s_{r}", tag="v1s", bufs=1)
        nc.vector.tensor_copy(p0s[:, 0:E], p0[:])
        nc.scalar.activation(p1s[:], p1[:], AF.Copy)
        nc.vector.tensor_copy(k0s[:, 0:E], k0[:])
        nc.scalar.activation(k1s[:], k1[:], AF.Copy)
        nc.vector.tensor_copy(v0s[:, 0:E], v0[:])
        nc.scalar.activation(v1s[:], v1[:], AF.Copy)

        # (ch,b) rows -> rs_in's (b,ch) rows, regrouped by the DMA pattern.
        # ch layout per batch: [P0 P1 P2 | Ck0..3 | Cv0..3].  The SBUF side
        # stays a plain 2-d partition walk (same element order as the 3-d
        # DRAM view; dma_start only requires equal sizes).
        rs_v = rs_in[:].rearrange("(b c) e -> c b e", c=NCH)
        for s_t, ch0, nch in ((p0s, 0, 2), (p1s, 2, 1), (k0s, 3, 2),
                              (k1s, 5, 2), (v0s, 7, 2), (v1s, 9, 2)):
            nc.sync.dma_start(rs_v[ch0:ch0 + nch], s_t[:, 0:E])

    nc.gpsimd.collective_compute(
        "ReduceScatter", ALU.add, replica_groups=rg,
        ins=[rs_in.opt()], outs=[rs_out.opt()],
    )
    if rep == 0 and "dbg_rs" in dbg_outs:
        nc.gpsimd.dma_start(dbg_outs["dbg_rs"][:], rs_out[:])
        nc.gpsimd.dma_start(dbg_outs["dbg_rsin"][:], rs_in[:])

    # all post-scatter rows in one DMA, channel-major so every per-batch
    # slice starts at partition 0 (engines cannot shift partition lanes);
    # emitted BEFORE the wot/xpall loads so it dispatches the moment the
    # ReduceScatter completes instead of queueing behind them
    rs_all = xpool.tile([NCH, BLOC * E], F16, name=f"rs_all_{r}",
                        tag="rs_all", bufs=1)
    nc.sync.dma_start(
        rs_all[:].rearrange("c (b e) -> c b e", b=BLOC),
        rs_out[:].rearrange("(b c) e -> c b e", c=NCH))

    # stage-C inputs, buffer-reusing the eviction tags (see above): the
    # WAR dependency releases them at the stage-A tail, so they stream
    # during the collective window
    wots = []
    for half, tag in ((0, "p0s"), (1, "k0s")):
        wo_t = xpool.tile([128, 2 * TLOC], F16, name=f"wo{half}_{r}",
                          tag=tag, bufs=1)
        nc.sync.dma_start(
            wo_t[:].rearrange("p (a t) -> p a t", a=2),
            wot[half * 256:(half + 1) * 256, :].rearrange(
                "(a p) t -> p a t", a=2))
        wots.append(wo_t)
    xpbig = xpool.tile([128, 2 * TLOC], F16, name=f"xpall_{r}", tag="v0s",
                       bufs=1)
    xpall = xpbig[0:96, :]
    nc.sync.dma_start(
        xpall.rearrange("p (m t) -> p m t", m=2),
        xp[:].rearrange("(m p) t -> p m t", m=2))

    # ---- stage B: per-batch attention ----------------------------------
    with tc.tile_pool(name="psB", space="PSUM", bufs=1) as psB, \
         tc.tile_pool(name="sbB", bufs=2) as sb:
        ag_s = sb.tile([128, EC * GW], F16, name=f"ag_s_{r}", tag="ag_s",
                       bufs=1)

        # per-batch state carried across the 1-batch software-pipeline skew
        qkv_sL = [None] * BLOC
        vt_sL = [None] * BLOC
        etL = [None] * BLOC
        apsL = [None] * BLOC

        def emit_mix(b):
            """channel mixes for batch b: Q and K/W3V/Z in separate PSUM
            tiles so the two DVE evictions (which also apply the biases)
            free their banks independently and pipeline with the consumers.
            qkv_s layout: columns 0..E-1 = Q (rows 0..15), E..2E-1 = K
            (rows 0..15) / W3V+ones (rows 32..35)."""
            rs_b = rs_all[:, b * E:(b + 1) * E]
            q_ps = psB.tile([HID, E], F32, name=f"qps{b}_{r}", tag="qps",
                            bufs=1)
            kv_ps = psB.tile([NKV, E], F32, name=f"kvps{b}_{r}", tag="kvps",
                             bufs=1)
            nc.tensor.matmul(q_ps[:], w12q_s[:], rs_b, start=True, stop=True)
            nc.tensor.matmul(kv_ps[:], w12kv_s[:], rs_b, start=True,
                             stop=True)
            qkv_s = sb.tile([NKV, 2 * E], F16, name=f"qkv_s{b}_{r}",
                            tag="qkv_s", bufs=2)
            nc.vector.tensor_tensor(out=qkv_s[0:HID, 0:E], in0=q_ps[:],
                                    in1=biasqkv_s[0:HID, 0:E], op=ALU.add)
            nc.vector.tensor_tensor(out=qkv_s[:, E:2 * E], in0=kv_ps[:],
                                    in1=biasqkv_s[:, E:2 * E], op=ALU.add)
            qkv_sL[b] = qkv_s

        def emit_vt(b):
            """(W3V | 1)^T: rows 32..35 of the K-half -> [128, 4] per
            f-chunk.  vtp and atp share PSUM banks (tag "tp")."""
            qkv_s = qkv_sL[b]
            vtp = psB.tile([128, EC * CH1Z], F16, name=f"vtp{b}_{r}",
                           tag="tp", bufs=1)
            for fc in range(EC):
                nc.tensor.transpose(
                    vtp[:, fc * CH1Z:(fc + 1) * CH1Z],
                    qkv_s[2 * HID:NKV, E + fc * 128:E + (fc + 1) * 128],
                    identt_s[2 * HID:NKV, :])
            vt_s = sb.tile([128, EC * CH1Z], F16, name=f"vt_s{b}_{r}",
                           tag="vt_s", bufs=2)
            nc.vector.tensor_copy(vt_s[:], vtp[:])
            vt_sL[b] = vt_s

        def emit_s(b):
            """four S^T chunk matmuls, one exp each (chunk granularity
            keeps the PE->ACT pipeline fine-grained)."""
            qkv_s = qkv_sL[b]
            q_ap = qkv_s[0:HID, 0:E]
            ets = []
            for fc in range(EC):
                sps = psB.tile([128, E], F32, name=f"sps{b}{fc}_{r}",
                               tag="s", bufs=3)
                nc.tensor.matmul(
                    sps[:], qkv_s[0:HID, E + fc * 128:E + (fc + 1) * 128],
                    q_ap, start=True, stop=True)
                et = sb.tile([128, E], F16, name=f"et{b}{fc}_{r}", tag="et",
                             bufs=8)
                nc.scalar.activation(et[:], sps[:], AF.Exp, scale=SCALE,
                                     bias=nshift[:])
                ets.append(et)
            etL[b] = ets

        def emit_attn(b):
            """attn matmuls for batch b (needs et[b] ready).

            Rows: 0..2 = W3-mixed attention (unnormalized), 3 = Z."""
            aps = psB.tile([CH1Z, E], F32, name=f"aps{b}_{r}", tag="attn",
                           bufs=1)
            vt_s, ets = vt_sL[b], etL[b]
            for fc in range(EC):
                nc.tensor.matmul(aps[:], vt_s[:, fc * CH1Z:(fc + 1) * CH1Z],
                                 ets[fc][:], start=(fc == 0), stop=(fc == 3))
            apsL[b] = aps

        def emit_back(b):
            """transpose attn rows into e-partition layout for the gather."""
            aps = apsL[b]
            an_s = sb.tile([CH1Z, E], F16, name=f"an_s{b}_{r}", tag="an_s",
                           bufs=2)
            nc.vector.tensor_copy(an_s[:], aps[:])
            atp = psB.tile([128, EC * CH1Z], F16, name=f"atp{b}_{r}",
                           tag="atp", bufs=1)
            for ec in range(EC):
                nc.tensor.transpose(
                    atp[:, ec * CH1Z:(ec + 1) * CH1Z],
                    an_s[:, ec * 128:(ec + 1) * 128],
                    identt_s[0:CH1Z, :])
            # ag_s column layout per e-chunk block: (b, [o0 o1 o2 Z])
            nc.vector.tensor_copy(
                ag_s[:].rearrange("p (c q) -> p c q", c=EC)[
                    :, :, b * CH1Z:(b + 1) * CH1Z],
                atp[:].rearrange("p (c k) -> p c k", c=EC))

        # software pipeline with a 1-batch skew: after each mix, the PE
        # chews on batch b-1 (attn + output transposes) while the DVE
        # evicts batch b's mixes, so no engine waits on the
        # mix->evict->S chain
        emit_mix(0)
        emit_vt(0)
        emit_s(0)
        for b in range(1, BLOC):
            emit_mix(b)
            emit_attn(b - 1)
            emit_back(b - 1)
            emit_vt(b)
            emit_s(b)
        emit_attn(BLOC - 1)
        emit_back(BLOC - 1)

        if rep == 0 and "dbg_q" in dbg_outs:
            nc.gpsimd.dma_start(dbg_outs["dbg_q"][:], qkv_sL[0][0:HID, 0:E])
            nc.gpsimd.dma_start(dbg_outs["dbg_kv"][:],
                                qkv_sL[0][:, E:2 * E])
        if rep == 0 and "dbg_et" in dbg_outs:
            nc.gpsimd.dma_start(dbg_outs["dbg_et"][:], etL[0][:])

        nc.sync.dma_start(
            ag_in[:].rearrange("(c p) w -> p c w", c=EC),
            ag_s[:].rearrange("p (c w) -> p c w", c=EC))

    nc.gpsimd.collective_compute(
        "AllGather", ALU.bypass, replica_groups=rg,
        ins=[ag_in.opt()], outs=[ag_out.opt()],
    )
    if rep == 0 and "dbg_ag" in dbg_outs:
        nc.gpsimd.dma_start(dbg_outs["dbg_ag"][:], ag_out[:])

    # ---- stage C: y^T[(b,o), t] = at3^T Wo^T + bias + residual ----------
    from concourse.bass import broadcast_tensor_aps

    with tc.tile_pool(name="psC", space="PSUM", bufs=1) as psC, \
         tc.tile_pool(name="sbC", bufs=1) as sc2:
        MH = CH1 * B // 2      # 96 (b,o) rows per M-half
        atall = sc2.tile([128, EC * NCORES * GW], F16, name=f"atall_{r}",
                         tag="atall", bufs=1)
        ag_v = ag_out[:].rearrange("(g c p) w -> c p g w", g=NCORES, c=EC)
        for ec in range(EC):
            nc.gpsimd.dma_start(
                atall[:, ec * NCORES * GW:(ec + 1) * NCORES * GW].rearrange(
                    "p (g w) -> p g w", g=NCORES),
                ag_v[ec])
        at3 = []
        for ec in range(EC):
            a_u = atall[:, ec * NCORES * GW:(ec + 1) * NCORES * GW]
            # normalize: at3[e, (g,b,o)] = att[e,(g,b,o)] * (1/Z[e,(g,b)])
            a_n = sc2.tile([128, CH1 * B], F16, name=f"at3{ec}_{r}",
                           tag=f"at3{ec}", bufs=1)
            u = a_u.rearrange("p (g b k) -> p g b k", g=NCORES, b=BLOC)
            zr = sc2.tile([128, B], F32, name=f"zr{ec}_{r}", tag=f"zr{ec}",
                          bufs=1)
            zr_v = zr[:].rearrange("p (g b one) -> p g b one", g=NCORES,
                                   one=1)
            nc.vector.reciprocal(zr_v, u[:, :, :, CH1:CH1Z])
            num, den = broadcast_tensor_aps(u[:, :, :, 0:CH1], zr_v)
            nc.vector.tensor_tensor(
                out=a_n[:].rearrange("p (g b k) -> p g b k", g=NCORES,
                                     b=BLOC),
                in0=num, in1=den, op=ALU.mult)
            at3.append(a_n)

        # yt row index is (b, o) = (g, w); M-halves split at g=4; each
        # half accumulates into one wide SBUF tile flushed by a single DMA
        for mh in range(2):
            c0 = mh * MH   # 0 or 96
            y_s = sc2.tile([MH, TLOC], F32, name=f"y_s{mh}_{r}",
                           tag=f"y_s{mh}", bufs=1)
            for m4 in range(EC):
                t0, t1 = m4 * 512, (m4 + 1) * 512
                yps = psC.tile([MH, 512], F32, name=f"yps{mh}{m4}_{r}",
                               tag="yps", bufs=8)
                # bias rank-1 first: it only needs constants, so it runs
                # during the AllGather window and keeps the PE warm
                nc.tensor.matmul(yps[:], w3sr_s[:, c0:c0 + MH],
                                 bot_s[:, t0:t1], start=True, stop=False)
                for ec in range(EC):
                    nc.tensor.matmul(
                        yps[:], at3[ec][:, c0:c0 + MH],
                        wots[ec // 2][:, (ec % 2) * TLOC + t0:
                                      (ec % 2) * TLOC + t1],
                        start=False, stop=(ec == EC - 1))
                nc.vector.tensor_tensor(
                    out=y_s[:, t0:t1], in0=yps[:],
                    in1=xpall[:, mh * TLOC + t0:mh * TLOC + t1], op=ALU.add)
            nc.sync.dma_start(yt[c0:c0 + MH, :], y_s[:])


_CACHE = {}


def _get_program(reps: int, dbg: bool = False):
    key = (reps, dbg)
    if key not in _CACHE:
        _CACHE[key] = build_program(reps, dbg=dbg)
    return _CACHE[key]


class _PjrtRunner:
    """jit-once wrapper around bass2jax so repeat calls skip recompile/reload."""

    def __init__(self, nc):
        import jax
        from jax.sharding import Mesh, PartitionSpec
        from jax.experimental.shard_map import shard_map
        from concourse import bass2jax

        bass2jax.install_neuronx_cc_hook()
        self.nc = nc
        in_names, out_names, out_avals, zero_outs = [], [], [], []
        partition_name = (nc.partition_id_tensor.name
                          if nc.partition_id_tensor else None)
        for alloc in nc.m.functions[0].allocations:
            if not isinstance(alloc, mybir.MemoryLocationSet):
                continue
            name = alloc.memorylocations[0].name
            if alloc.kind == "ExternalInput":
                if name != partition_name:
                    in_names.append(name)
            elif alloc.kind == "ExternalOutput":
                shape = tuple(alloc.tensor_shape)
                dtype = mybir.dt.np(alloc.dtype)
                out_names.append(name)
                out_avals.append(jax.core.ShapedArray(shape, dtype))
                zero_outs.append(np.zeros(shape, dtype))
        self.n_params = len(in_names)
        self.in_names = list(in_names)
        self.out_names = out_names
        self.out_avals = out_avals
        self.zero_outs = zero_outs
        all_in_names = in_names + out_names
        if partition_name is not None:
            all_in_names.append(partition_name)

        n_outs = len(out_names)
        donate = tuple(range(self.n_params, self.n_params + n_outs))

        def _body(*args):
            operands = list(args)
            if partition_name is not None:
                operands.append(bass2jax.partition_id_tensor())
            outs = bass2jax._bass_exec_p.bind(
                *operands,
                out_avals=tuple(out_avals),
                in_names=tuple(all_in_names),
                out_names=tuple(out_names),
                lowering_input_output_aliases=(),
                sim_require_finite=True,
                sim_require_nnan=True,
                nc=nc,
            )
            return tuple(outs)

        devices = jax.devices()[:NCORES]
        mesh = Mesh(np.asarray(devices), ("core",))
        self.mesh = mesh
        in_specs = (PartitionSpec("core"),) * (self.n_params + n_outs)
        out_specs = (PartitionSpec("core"),) * n_outs
        self.fn = jax.jit(
            shard_map(_body, mesh=mesh, in_specs=in_specs,
                      out_specs=out_specs, check_rep=False),
            donate_argnums=donate, keep_unused=True)

    def __call__(self, in_maps):
        concat_in = [
            np.concatenate([np.asarray(in_maps[c][nm]) for c in range(NCORES)],
                           axis=0)
            for nm in self.in_names]
        concat_zeros = [
            np.zeros((NCORES * z.shape[0], *z.shape[1:]), z.dtype)
            for z in self.zero_outs]
        out_arrs = self.fn(*concat_in, *concat_zeros)
        return [
            {nm: np.asarray(out_arrs[i]).reshape(
                NCORES, *self.out_avals[i].shape)[c]
             for i, nm in enumerate(self.out_names)}
            for c in range(NCORES)]


_RUNNERS = {}


def _get_runner(reps: int, dbg: bool = False):
    key = (reps, dbg)
    if key not in _RUNNERS:
        _RUNNERS[key] = _PjrtRunner(_get_program(reps, dbg=dbg))
    return _RUNNERS[key]


def make_in_maps(x, W1, W2, Wq, bq, Wk, bk, Wv, bv, Wo, bo, W3):
    """Host-side sharding: slicing / transposition / constant assembly only."""
    f32, f16 = np.float32, np.float16
    x = np.asarray(x, f32)

    # Q mix: rows = the 11 reduced channels, cols = 16 Q outputs
    w12q = np.zeros((NCH, HID), f16)
    w12q[0:CH1, :] = np.asarray(W1, f32).T
    # K/(W3 V) mix: K -> cols 0..15; W3-folded V -> cols 32..34 (parked at
    # partition base 32 so its PE transpose is tile-position aligned);
    # col 35 = Z-ones row (filled by the bias plane)
    w3w2 = np.asarray(W3, f32) @ np.asarray(W2, f32)     # [3, 4]
    w12kv = np.zeros((NCH, NKV), f16)
    w12kv[CH1:CH1 + CH2, 0:HID] = np.asarray(W2, f32).T
    w12kv[CH1 + CH2:NCH, 2 * HID:2 * HID + CH1] = w3w2.T

    w3sum = np.asarray(W3, f32).sum(axis=1)              # [3]
    biasqkv = np.zeros((NKV, 2 * E), f32)
    biasqkv[0:HID, 0:E] = np.asarray(bq, f32)[None, :]
    biasqkv[0:HID, E:2 * E] = np.asarray(bk, f32)[None, :]
    biasqkv[2 * HID:2 * HID + CH1, E:2 * E] = (
        w3sum[:, None] * np.asarray(bv, f32)[None, :])
    biasqkv[NKV - 1, E:2 * E] = 1.0

    identt = np.zeros((NKV, CH1Z), f16)
    identt[0:CH1Z, :] = np.eye(CH1Z, dtype=f16)
    identt[2 * HID:NKV, :] = np.eye(CH1Z, dtype=f16)

    w3sr = np.tile(w3sum, B)[None, :].astype(f16)        # [1, 192], b*3+o

    in_maps = []
    for c in range(NCORES):
        sl = slice(c * TLOC, (c + 1) * TLOC)
        xt = np.ascontiguousarray(
            np.transpose(x[:, :, sl], (2, 1, 0)).reshape(TLOC, 7 * B))
        m = {
            "xt": xt.astype(f16),
            "xp": np.ascontiguousarray(
                x[:, :CH1, sl].reshape(CH1 * B, TLOC)).astype(f16),
            "wqkv": np.concatenate(
                [np.asarray(Wq, f32)[:, sl].T, np.asarray(Wk, f32)[:, sl].T,
                 np.asarray(Wv, f32)[:, sl].T], axis=1).astype(f16),
            "wot": np.asarray(Wo, f32)[sl, :].T.astype(f16),
            "bot": np.asarray(bo, f32)[sl][None, :].astype(f16),
            "w12q": w12q, "w12kv": w12kv, "biasqkv": biasqkv,
            "identt": identt, "w3sr": w3sr,
        }
        in_maps.append(m)
    return in_maps


def assemble_output(results):
    """[per-core yt [192, 2048]] -> [B, CH1, T]; row = b*CH1 + o."""
    arr = np.stack([res["yt"] for res in results], axis=0)  # [8, 192, 2048]
    return np.ascontiguousarray(
        arr.transpose(1, 0, 2).reshape(B, CH1, T))


def run(inputs, reps: int = 1, dbg: bool = False):
    runner = _get_runner(reps, dbg=dbg)
    in_maps = make_in_maps(**inputs)
    results = runner(in_maps)
    if dbg:
        return assemble_output(results), results
    return assemble_output(results)


def kernel(**inputs) -> np.ndarray:
    return run(inputs, reps=1)


def time_reps(inputs, reps: int, n: int = 10):
    """Per-call wall times with device-resident inputs (first call = warmup)."""
    import time
    import jax
    from jax.sharding import NamedSharding, PartitionSpec

    runner = _get_runner(reps)
    in_maps = make_in_maps(**inputs)
    concat = [
        np.concatenate([np.asarray(in_maps[c][nm]) for c in range(NCORES)],
                       axis=0)
        for nm in runner.in_names]
    sh = NamedSharding(runner.mesh, PartitionSpec("core"))
    dev = [jax.device_put(a, sh) for a in concat]
    times = []
    for i in range(n + 1):
        zeros = [np.zeros((NCORES * z.shape[0], *z.shape[1:]), z.dtype)
                 for z in runner.zero_outs]
        t0 = time.perf_counter()
        out = runner.fn(*dev, *zeros)
        jax.block_until_ready(out)
        dt = time.perf_counter() - t0
        if i > 0:
            times.append(dt)
    return times


# revision 8
# speedup vs baseline: 1.0416x; 1.0110x over previous
"""Trainium2 Bass kernel for nn_AttnResBlock (B=64, CH1=3, CH2=4, HID=16, T=16384, E=512).

Strategy: tensor-parallel split of the T dimension across 8 cores, fp16
datapath for every large tensor.

  y = p + W3 @ (attn(W1@p, W2@c | Wq,Wk,Wv) @ Wo)        p = x[:,:3], c = x[:,3:7]

The big weights (Wq/Wk/Wv [E,T], Wo [T,E]) dominate memory traffic.  Each
core owns a T-slice of 2048 and reads only its slice of each projection
weight; all large streams are fp16 (host-rounded), halving HBM bytes vs
fp32 while the PE accumulates in fp32 PSUM (matmul error ~1e-3 rel, well
inside the 2e-2 gate).

  stage A (T-parallel):   P[b,c,e]     = sum_{t in Ti} p[b,c,t] Wq[e,t]   (partial)
                          Ck/Cv[b,c,e] = sum_{t in Ti} c[b,c,t] W{k,v}[e,t]
    11 rows/batch (3 P + 4 Ck + 4 Cv) go through a fp16 ReduceScatter;
    core r receives batches 8r..8r+7.  Channel mixes and biases are applied
    AFTER the reduce (biases via a precomputed [49,E] bias plane added by
    DVE, so they ride along with the PSUM->SBUF eviction).
  stage B (B-parallel):   two block-diag matmuls produce Q [16,E] and K/V
                          [49,E] per batch (V parked at partition base 32 so
                          the V^T PE transposes are tile-position legal);
                          V^T via 4 tiny PE transposes (col 16 of each chunk
                          = 1.0 from the bias plane, which makes the attn
                          matmul also emit the softmax partition sums Z as
                          row 17); S^T = K^T Q, exp on ACT (fp16 out),
                          attn = V^T @ exp(S^T), W3-mix, 1/Z normalize.
                          Normalized attn3[e,(b,o)] is AllGather'ed
                          (tiny: 25KB/core).
  stage C (T-parallel):   y^T[(b,o), t] = at3^T Wo-slice contraction over e,
                          + bo*w3sum rank-1 term, + residual p (fp32 xp,
                          loaded during the collective window).

DMA instruction count is kept low (HWDGE dispatch costs ~625ns/DMA): weights
stream in 2-k-tile chunks, the ReduceScatter staging is 6 DMAs with
(ch,b)->(b,ch) row regrouping done by the DMA access pattern, and stage B
reads all of its post-scatter rows in a single DMA.
"""

import numpy as np

import concourse.bacc as bacc
import concourse.tile as tile
import concourse.mybir as mybir

F32 = mybir.dt.float32
F16 = mybir.dt.float16
AF = mybir.ActivationFunctionType
ALU = mybir.AluOpType

B, CH1, CH2, HID, T, E = 64, 3, 4, 16, 16384, 512
NCORES = 8
TLOC = T // NCORES          # 2048
KT = TLOC // 128            # 16 k-tiles in stage A
KCH = 4                     # k-tiles per DMA chunk
NCHUNK = KT // KCH          # 8 chunks
EC = E // 128               # 4 e/f chunks
BLOC = B // NCORES          # 8 batches per core in stage B
NCH = CH1 + CH2 + CH2       # 11 rows/batch through the ReduceScatter
NKV = 2 * HID + CH1 + 1     # 36 KV-mix rows: K 0..15, pad, W3V 32..34, Z 35
SCALE = 1.0 / np.sqrt(HID)  # attention scale
EXPSHIFT = 10.0             # exp(S*SCALE - 10): keeps exp inside fp16 range;
                            # softmax is shift-invariant so Z divides it out
CH1Z = CH1 + 1              # 3 W3-mixed channels + the softmax sum Z
GW = CH1Z * BLOC            # 32 AllGather columns per rank: (b, [o0 o1 o2 Z])


def build_program(reps: int = 1, dbg: bool = False):
    nc = bacc.Bacc("TRN2", target_bir_lowering=False, debug=False,
                   num_devices=NCORES)

    xt = nc.dram_tensor("xt", [TLOC, 7 * B], F16, kind="ExternalInput")
    xp = nc.dram_tensor("xp", [CH1 * B, TLOC], F16, kind="ExternalInput")
    wqkv = nc.dram_tensor("wqkv", [TLOC, 3 * E], F16, kind="ExternalInput")
    wot = nc.dram_tensor("wot", [E, TLOC], F16, kind="ExternalInput")
    bot = nc.dram_tensor("bot", [1, TLOC], F16, kind="ExternalInput")
    w12q = nc.dram_tensor("w12q", [NCH, HID], F16, kind="ExternalInput")
    w12kv = nc.dram_tensor("w12kv", [NCH, NKV], F16, kind="ExternalInput")
    biasqkv = nc.dram_tensor("biasqkv", [NKV, 2 * E], F32,
                             kind="ExternalInput")
    identt = nc.dram_tensor("identt", [NKV, CH1Z], F16,
                            kind="ExternalInput")
    w3sr = nc.dram_tensor("w3sr", [1, CH1 * B], F16, kind="ExternalInput")
    yt = nc.dram_tensor("yt", [CH1 * B, TLOC], F32, kind="ExternalOutput")

    rg = [list(range(NCORES))]

    with tile.TileContext(nc) as tc:
        with tc.tile_pool(name="const", bufs=1) as cp, \
             tc.tile_pool(name="dram", space="DRAM", bufs=1) as dp, \
             tc.tile_pool(name="xp", bufs=1) as xpool, \
             tc.tile_pool(name="wotp", bufs=1) as wotp:

            # small constants via the Pool (SWDGE) queue so they never
            # contend with the stage-A weight stream on HWDGE
            w12q_s = cp.tile([NCH, HID], F16, name="w12q_s")
            w12kv_s = cp.tile([NCH, NKV], F16, name="w12kv_s")
            biasqkv_s = cp.tile([NKV, 2 * E], F32, name="biasqkv_s")
            identt_s = cp.tile([NKV, CH1Z], F16, name="identt_s")
            w3sr_s = cp.tile([1, CH1 * B], F16, name="w3sr_s")
            bot_s = cp.tile([1, TLOC], F16, name="bot_s")
            nshift = cp.tile([128, 1], F32, name="nshift")
            nc.vector.memset(nshift[:], -EXPSHIFT)
            nc.gpsimd.dma_start(w12q_s[:], w12q[:])
            nc.gpsimd.dma_start(w12kv_s[:], w12kv[:])
            nc.gpsimd.dma_start(biasqkv_s[:], biasqkv[:])
            nc.gpsimd.dma_start(identt_s[:], identt[:])
            nc.gpsimd.dma_start(w3sr_s[:], w3sr[:])
            nc.gpsimd.dma_start(bot_s[:], bot[:])

            dbg_outs = {}
            if dbg:
                dbg_outs["dbg_rs"] = nc.dram_tensor(
                    "dbg_rs", [BLOC * NCH, E], F16, kind="ExternalOutput")
                dbg_outs["dbg_rsin"] = nc.dram_tensor(
                    "dbg_rsin", [B * NCH, E], F16, kind="ExternalOutput")
                dbg_outs["dbg_q"] = nc.dram_tensor(
                    "dbg_q", [HID, E], F16, kind="ExternalOutput")
                dbg_outs["dbg_kv"] = nc.dram_tensor(
                    "dbg_kv", [NKV, E], F16, kind="ExternalOutput")
                dbg_outs["dbg_et"] = nc.dram_tensor(
                    "dbg_et", [128, EC * E], F16, kind="ExternalOutput")
                dbg_outs["dbg_ag"] = nc.dram_tensor(
                    "dbg_ag", [NCORES * E, GW], F16, kind="ExternalOutput")
            for rep in range(reps):
                build_rep(nc, tc, dp, xpool, wotp, rep, rg, locals())

    nc.compile()
    return nc


def build_rep(nc, tc, dp, xpool, wotp, rep, rg, env):
    xt, xp, wqkv, wot, yt = (env[k] for k in
                             ("xt", "xp", "wqkv", "wot", "yt"))
    w12q_s, w12kv_s, biasqkv_s, identt_s = (
        env[k] for k in ("w12q_s", "w12kv_s", "biasqkv_s", "identt_s"))
    w3sr_s, bot_s, nshift = (env[k] for k in ("w3sr_s", "bot_s", "nshift"))
    dbg_outs = env.get("dbg_outs", {})

    r = f"r{rep}"

    # ---- DRAM bounce buffers for the collectives -------------------------
    rs_in = dp.tile([B * NCH, E], F16, name=f"rs_in_{r}", tag="rs_in", bufs=1)
    rs_out = dp.tile([BLOC * NCH, E], F16, name=f"rs_out_{r}", tag="rs_out",
                     bufs=1)
    ag_in = dp.tile([E, GW], F16, name=f"ag_in_{r}", tag="ag_in", bufs=1)
    ag_out = dp.tile([NCORES * E, GW], F16, name=f"ag_out_{r}", tag="ag_out",
                     bufs=1)

    # ---- stage A: big T-contraction ------------------------------------
    # x and the host-concatenated [wq|wk|wv] stream in k-tile chunks (two
    # HWDGE dispatches per chunk); small chunks first so the PE starts fast
    CHUNKS = (1, 1, 1, 1, 2, 2, 4, 4)
    assert sum(CHUNKS) == KT
    xts, wws = [], []
    k0c = 0
    for c, kch in enumerate(CHUNKS):
        xc = xpool.tile([128, kch * 7 * B], F16, name=f"x{c}_{r}",
                        tag=f"x{c}", bufs=1)
        wc = xpool.tile([128, kch * 3 * E], F16, name=f"ww{c}_{r}",
                        tag=f"ww{c}", bufs=1)
        rows = slice(k0c * 128, (k0c + kch) * 128)
        nc.sync.dma_start(
            xc[:].rearrange("p (a w) -> p a w", a=kch),
            xt[rows, :].rearrange("(a p) w -> p a w", a=kch))
        nc.sync.dma_start(
            wc[:].rearrange("p (a e) -> p a e", a=kch),
            wqkv[rows, :].rearrange("(a p) e -> p a e", a=kch))
        xts.append(xc)
        wws.append(wc)
        k0c += kch

    with tc.tile_pool(name="psA", space="PSUM", bufs=1) as psA, \
         tc.tile_pool(name="stgA", bufs=1) as sa:
        p0 = psA.tile([128, E], F32, name=f"p0_{r}", tag="p0", bufs=1)
        p1 = psA.tile([64, E], F32, name=f"p1_{r}", tag="p1", bufs=1)
        k0 = psA.tile([128, E], F32, name=f"k0_{r}", tag="k0", bufs=1)
        k1 = psA.tile([128, E], F32, name=f"k1_{r}", tag="k1", bufs=1)
        v0 = psA.tile([128, E], F32, name=f"v0_{r}", tag="v0", bufs=1)
        v1 = psA.tile([128, E], F32, name=f"v1_{r}", tag="v1", bufs=1)

        k0c = 0
        for c, kch in enumerate(CHUNKS):
            xc, wc = xts[c], wws[c]
            for a in range(kch):
                k = k0c + a
                st, sp = (k == 0), (k == KT - 1)
                x0 = a * 7 * B
                w0 = a * 3 * E
                wq_a = wc[:, w0:w0 + E]
                wk_a = wc[:, w0 + E:w0 + 2 * E]
                wv_a = wc[:, w0 + 2 * E:w0 + 3 * E]
                nc.tensor.matmul(p0[:], xc[:, x0:x0 + 128], wq_a,
                                 start=st, stop=sp)
                nc.tensor.matmul(p1[:], xc[:, x0 + 128:x0 + 192], wq_a,
                                 start=st, stop=sp)
                nc.tensor.matmul(k0[:], xc[:, x0 + 192:x0 + 320], wk_a,
                                 start=st, stop=sp)
                nc.tensor.matmul(v0[:], xc[:, x0 + 192:x0 + 320], wv_a,
                                 start=st, stop=sp)
                nc.tensor.matmul(k1[:], xc[:, x0 + 320:x0 + 448], wk_a,
                                 start=st, stop=sp)
                nc.tensor.matmul(v1[:], xc[:, x0 + 320:x0 + 448], wv_a,
                                 start=st, stop=sp)
            k0c += kch

        # PSUM -> SBUF fp16, split across the two PSUM-capable engines.
        # p0s/k0s/v0s live in the rep-long xpool, sized [128, 2*TLOC] (only
        # [:, :E] used) so the wot/xpall tiles below reuse their buffers:
        # the tag-rotation WAR dependency delays those big loads until the
        # staging DMAs have read the evictions — i.e. into the collective
        # window — instead of stealing stage-A DMA bandwidth.
        p0s = xpool.tile([128, 2 * TLOC], F16, name=f"p0s_{r}", tag="p0s",
                         bufs=1)
        p1s = sa.tile([64, E], F16, name=f"p1s_{r}", tag="p1s", bufs=1)
        k0s = xpool.tile([128, 2 * TLOC], F16, name=f"k0s_{r}", tag="k0s",
                         bufs=1)
        k1s = sa.tile([128, E], F16, name=f"k1s_{r}", tag="k1s", bufs=1)
        v0s = xpool.tile([128, 2 * TLOC], F16, name=f"v0s_{r}", tag="v0s",
                         bufs=1)
        v1s = sa.tile([128, E], F16, name=f"v1s_{r}", tag="v1s", bufs=1)
        nc.vector.tensor_copy(p0s[:, 0:E], p0[:])
        nc.scalar.activation(p1s[:], p1[:], AF.Copy)
        nc.vector.tensor_copy(k0s[:, 0:E], k0[:])
        nc.scalar.activation(k1s[:], k1[:], AF.Copy)
        nc.vector.tensor_copy(v0s[:, 0:E], v0[:])
        nc.scalar.activation(v1s[:], v1[:], AF.Copy)

        # (ch,b) rows -> rs_in's (b,ch) rows, regrouped by the DMA pattern.
        # ch layout per batch: [P0 P1 P2 | Ck0..3 | Cv0..3].  The SBUF side
        # stays a plain 2-d partition walk (same element order as the 3-d
        # DRAM view; dma_start only requires equal sizes).
        rs_v = rs_in[:].rearrange("(b c) e -> c b e", c=NCH)
        for s_t, ch0, nch in ((p0s, 0, 2), (p1s, 2, 1), (k0s, 3, 2),
                              (k1s, 5, 2), (v0s, 7, 2), (v1s, 9, 2)):
            nc.sync.dma_start(rs_v[ch0:ch0 + nch], s_t[:, 0:E])

    nc.gpsimd.collective_compute(
        "ReduceScatter", ALU.add, replica_groups=rg,
        ins=[rs_in.opt()], outs=[rs_out.opt()],
    )
    if rep == 0 and "dbg_rs" in dbg_outs:
        nc.gpsimd.dma_start(dbg_outs["dbg_rs"][:], rs_out[:])
        nc.gpsimd.dma_start(dbg_outs["dbg_rsin"][:], rs_in[:])

    # all post-scatter rows in one DMA, channel-major so every per-batch
    # slice starts at partition 0 (engines cannot shift partition lanes);
    # emitted BEFORE the wot/xpall loads so it dispatches the moment the
    # ReduceScatter completes instead of queueing behind them
    rs_all = xpool.tile([NCH, BLOC * E], F16, name=f"rs_all_{r}",
                        tag="rs_all", bufs=1)
    nc.sync.dma_start(
        rs_all[:].rearrange("c (b e) -> c b e", b=BLOC),
        rs_out[:].rearrange("(b c) e -> c b e", c=NCH))

    # stage-C inputs, buffer-reusing the eviction tags (see above): the
    # WAR dependency releases them at the stage-A tail, so they stream
    # during the collective window
    wots = []
    for half, tag in ((0, "p0s"), (1, "k0s")):
        wo_t = xpool.tile([128, 2 * TLOC], F16, name=f"wo{half}_{r}",
                          tag=tag, bufs=1)
        nc.sync.dma_start(
            wo_t[:].rearrange("p (a t) -> p a t", a=2),
            wot[half * 256:(half + 1) * 256, :].rearrange(
                "(a p) t -> p a t", a=2))
        wots.append(wo_t)
    xpbig = xpool.tile([128, 2 * TLOC], F16, name=f"xpall_{r}", tag="v0s",
                       bufs=1)
    xpall = xpbig[0:96, :]
    nc.sync.dma_start(
        xpall.rearrange("p (m t) -> p m t", m=2),
        xp[:].rearrange("(m p) t -> p m t", m=2))

    # ---- stage B: per-batch attention ----------------------------------
    with tc.tile_pool(name="psB", space="PSUM", bufs=1) as psB, \
         tc.tile_pool(name="sbB", bufs=2) as sb:
        ag_s = sb.tile([128, EC * GW], F16, name=f"ag_s_{r}", tag="ag_s",
                       bufs=1)

        # per-batch state carried across the 1-batch software-pipeline skew
        qkv_sL = [None] * BLOC
        vt_sL = [None] * BLOC
        etL = [None] * BLOC
        apsL = [None] * BLOC

        def emit_mix(b):
            """channel mixes for batch b: Q and K/W3V/Z in separate PSUM
            tiles so the two DVE evictions (which also apply the biases)
            free their banks independently and pipeline with the consumers.
            qkv_s layout: columns 0..E-1 = Q (rows 0..15), E..2E-1 = K
            (rows 0..15) / W3V+ones (rows 32..35)."""
            rs_b = rs_all[:, b * E:(b + 1) * E]
            q_ps = psB.tile([HID, E], F32, name=f"qps{b}_{r}", tag="qps",
                            bufs=1)
            kv_ps = psB.tile([NKV, E], F32, name=f"kvps{b}_{r}", tag="kvps",
                             bufs=1)
            nc.tensor.matmul(q_ps[:], w12q_s[:], rs_b, start=True, stop=True)
            nc.tensor.matmul(kv_ps[:], w12kv_s[:], rs_b, start=True,
                             stop=True)
            qkv_s = sb.tile([NKV, 2 * E], F16, name=f"qkv_s{b}_{r}",
                            tag="qkv_s", bufs=2)
            nc.vector.tensor_tensor(out=qkv_s[0:HID, 0:E], in0=q_ps[:],
                                    in1=biasqkv_s[0:HID, 0:E], op=ALU.add)
            nc.vector.tensor_tensor(out=qkv_s[:, E:2 * E], in0=kv_ps[:],
                                    in1=biasqkv_s[:, E:2 * E], op=ALU.add)
            qkv_sL[b] = qkv_s

        def emit_vt(b):
            """(W3V | 1)^T: rows 32..35 of the K-half -> [128, 4] per
            f-chunk.  vtp and atp share PSUM banks (tag "tp")."""
            qkv_s = qkv_sL[b]
            vtp = psB.tile([128, EC * CH1Z], F16, name=f"vtp{b}_{r}",
                           tag="tp", bufs=1)
            for fc in range(EC):
                nc.tensor.transpose(
                    vtp[:, fc * CH1Z:(fc + 1) * CH1Z],
                    qkv_s[2 * HID:NKV, E + fc * 128:E + (fc + 1) * 128],
                    identt_s[2 * HID:NKV, :])
            vt_s = sb.tile([128, EC * CH1Z], F16, name=f"vt_s{b}_{r}",
                           tag="vt_s", bufs=2)
            nc.vector.tensor_copy(vt_s[:], vtp[:])
            vt_sL[b] = vt_s

        def emit_s(b):
            """four S^T chunk matmuls, one exp each (chunk granularity
            keeps the PE->ACT pipeline fine-grained)."""
            qkv_s = qkv_sL[b]
            q_ap = qkv_s[0:HID, 0:E]
            ets = []
            for fc in range(EC):
                sps = psB.tile([128, E], F32, name=f"sps{b}{fc}_{r}",
                               tag="s", bufs=3)
                nc.tensor.matmul(
                    sps[:], qkv_s[0:HID, E + fc * 128:E + (fc + 1) * 128],
                    q_ap, start=True, stop=True)
                et = sb.tile([128, E], F16, name=f"et{b}{fc}_{r}", tag="et",
                             bufs=8)
                nc.scalar.activation(et[:], sps[:], AF.Exp, scale=SCALE,
                                     bias=nshift[:])
                ets.append(et)
            etL[b] = ets

        def emit_attn(b):
            """attn matmuls for batch b (needs et[b] ready).

            Rows: 0..2 = W3-mixed attention (unnormalized), 3 = Z."""
            aps = psB.tile([CH1Z, E], F32, name=f"aps{b}_{r}", tag="attn",
                           bufs=1)
            vt_s, ets = vt_sL[b], etL[b]
            for fc in range(EC):
                nc.tensor.matmul(aps[:], vt_s[:, fc * CH1Z:(fc + 1) * CH1Z],
                                 ets[fc][:], start=(fc == 0), stop=(fc == 3))
            apsL[b] = aps

        def emit_back(b):
            """transpose attn rows into e-partition layout for the gather."""
            aps = apsL[b]
            an_s = sb.tile([CH1Z, E], F16, name=f"an_s{b}_{r}", tag="an_s",
                           bufs=2)
            nc.vector.tensor_copy(an_s[:], aps[:])
            atp = psB.tile([128, EC * CH1Z], F16, name=f"atp{b}_{r}",
                           tag="atp", bufs=1)
            for ec in range(EC):
                nc.tensor.transpose(
                    atp[:, ec * CH1Z:(ec + 1) * CH1Z],
                    an_s[:, ec * 128:(ec + 1) * 128],
                    identt_s[0:CH1Z, :])
            # ag_s column layout per e-chunk block: (b, [o0 o1 o2 Z])
            nc.vector.tensor_copy(
                ag_s[:].rearrange("p (c q) -> p c q", c=EC)[
                    :, :, b * CH1Z:(b + 1) * CH1Z],
                atp[:].rearrange("p (c k) -> p c k", c=EC))

        # software pipeline with a 1-batch skew: after each mix, the PE
        # chews on batch b-1 (attn + output transposes) while the DVE
        # evicts batch b's mixes, so no engine waits on the
        # mix->evict->S chain
        emit_mix(0)
        emit_vt(0)
        emit_s(0)
        for b in range(1, BLOC):
            emit_mix(b)
            emit_attn(b - 1)
            emit_back(b - 1)
            emit_vt(b)
            emit_s(b)
        emit_attn(BLOC - 1)
        emit_back(BLOC - 1)

        if rep == 0 and "dbg_q" in dbg_outs:
            nc.gpsimd.dma_start(dbg_outs["dbg_q"][:], qkv_sL[0][0:HID, 0:E])
            nc.gpsimd.dma_start(dbg_outs["dbg_kv"][:],
                                qkv_sL[0][:, E:2 * E])
        if rep == 0 and "dbg_et" in dbg_outs:
            nc.gpsimd.dma_start(dbg_outs["dbg_et"][:], etL[0][:])

        nc.sync.dma_start(
            ag_in[:].rearrange("(c p) w -> p c w", c=EC),
            ag_s[:].rearrange("p (c w) -> p c w", c=EC))

    nc.gpsimd.collective_compute(
        "AllGather", ALU.bypass, replica_groups=rg,
        ins=[ag_in.opt()], outs=[ag_out.opt()],
    )
    if rep == 0 and "dbg_ag" in dbg_outs:
        nc.gpsimd.dma_start(dbg_outs["dbg_ag"][:], ag_out[:])

    # ---- stage C: y^T[(b,o), t] = at3^T Wo^T + bias + residual ----------
    from concourse.bass import broadcast_tensor_aps

    with tc.tile_pool(name="psC", space="PSUM", bufs=1) as psC, \
         tc.tile_pool(name="sbC", bufs=1) as sc2:
        MH = CH1 * B // 2      # 96 (b,o) rows per M-half
        atall = sc2.tile([128, EC * NCORES * GW], F16, name=f"atall_{r}",
                         tag="atall", bufs=1)
        ag_v = ag_out[:].rearrange("(g c p) w -> c p g w", g=NCORES, c=EC)
        for ec in range(EC):
            nc.gpsimd.dma_start(
                atall[:, ec * NCORES * GW:(ec + 1) * NCORES * GW].rearrange(
                    "p (g w) -> p g w", g=NCORES),
                ag_v[ec])
        at3 = []
        for ec in range(EC):
            a_u = atall[:, ec * NCORES * GW:(ec + 1) * NCORES * GW]
            # normalize: at3[e, (g,b,o)] = att[e,(g,b,o)] * (1/Z[e,(g,b)])
            a_n = sc2.tile([128, CH1 * B], F16, name=f"at3{ec}_{r}",
                           tag=f"at3{ec}", bufs=1)
            u = a_u.rearrange("p (g b k) -> p g b k", g=NCORES, b=BLOC)
            zr = sc2.tile([128, B], F32, name=f"zr{ec}_{r}", tag=f"zr{ec}",
                          bufs=1)
            zr_v = zr[:].rearrange("p (g b one) -> p g b one", g=NCORES,
                                   one=1)
            nc.vector.reciprocal(zr_v, u[:, :, :, CH1:CH1Z])
            num, den = broadcast_tensor_aps(u[:, :, :, 0:CH1], zr_v)
            nc.vector.tensor_tensor(
                out=a_n[:].rearrange("p (g b k) -> p g b k", g=NCORES,
                                     b=BLOC),
                in0=num, in1=den, op=ALU.mult)
            at3.append(a_n)

        # yt row index is (b, o) = (g, w); M-halves split at g=4; each
        # half accumulates into one wide SBUF tile flushed by a single DMA
        for mh in range(2):
            c0 = mh * MH   # 0 or 96
            y_s = sc2.tile([MH, TLOC], F32, name=f"y_s{mh}_{r}",
                           tag=f"y_s{mh}", bufs=1)
            for m4 in range(EC):
                t0, t1 = m4 * 512, (m4 + 1) * 512
                yps = psC.tile([MH, 512], F32, name=f"yps{mh}{m4}_{r}",
                               tag="yps", bufs=8)
                # bias rank-1 first: it only needs constants, so it runs
                # during the AllGather window and keeps the PE warm
                nc.tensor.matmul(yps[:], w3sr_s[:, c0:c0 + MH],
                                 bot_s[:, t0:t1], start=True, stop=False)
                for ec in range(EC):
                    nc.tensor.matmul(
                        yps[:], at3[ec][:, c0:c0 + MH],
                        wots[ec // 2][:, (ec % 2) * TLOC + t0:
                                      (ec % 2) * TLOC + t1],
                        start=False, stop=(ec == EC - 1))
                nc.vector.tensor_tensor(
                    out=y_s[:, t0:t1], in0=yps[:],
                    in1=xpall[:, mh * TLOC + t0:mh * TLOC + t1], op=ALU.add)
                if m4 % 2 == 1:
                    # flush each 1024-column half-pair as soon as its adds
                    # land so the final write drains only 384KB
                    nc.sync.dma_start(
                        yt[c0:c0 + MH, m4 * 512 - 512:m4 * 512 + 512],
                        y_s[:, m4 * 512 - 512:m4 * 512 + 512])


_CACHE = {}


def _get_program(reps: int, dbg: bool = False):
    key = (reps, dbg)
    if key not in _CACHE:
        _CACHE[key] = build_program(reps, dbg=dbg)
    return _CACHE[key]


class _PjrtRunner:
    """jit-once wrapper around bass2jax so repeat calls skip recompile/reload."""

    def __init__(self, nc):
        import jax
        from jax.sharding import Mesh, PartitionSpec
        from jax.experimental.shard_map import shard_map
        from concourse import bass2jax

        bass2jax.install_neuronx_cc_hook()
        self.nc = nc
        in_names, out_names, out_avals, zero_outs = [], [], [], []
        partition_name = (nc.partition_id_tensor.name
                          if nc.partition_id_tensor else None)
        for alloc in nc.m.functions[0].allocations:
            if not isinstance(alloc, mybir.MemoryLocationSet):
                continue
            name = alloc.memorylocations[0].name
            if alloc.kind == "ExternalInput":
                if name != partition_name:
                    in_names.append(name)
            elif alloc.kind == "ExternalOutput":
                shape = tuple(alloc.tensor_shape)
                dtype = mybir.dt.np(alloc.dtype)
                out_names.append(name)
                out_avals.append(jax.core.ShapedArray(shape, dtype))
                zero_outs.append(np.zeros(shape, dtype))
        self.n_params = len(in_names)
        self.in_names = list(in_names)
        self.out_names = out_names
        self.out_avals = out_avals
        self.zero_outs = zero_outs
        all_in_names = in_names + out_names
        if partition_name is not None:
            all_in_names.append(partition_name)

        n_outs = len(out_names)
        donate = tuple(range(self.n_params, self.n_params + n_outs))

        def _body(*args):
            operands = list(args)
            if partition_name is not None:
                operands.append(bass2jax.partition_id_tensor())
            outs = bass2jax._bass_exec_p.bind(
                *operands,
                out_avals=tuple(out_avals),
                in_names=tuple(all_in_names),
                out_names=tuple(out_names),
                lowering_input_output_aliases=(),
                sim_require_finite=True,
                sim_require_nnan=True,
                nc=nc,
            )
            return tuple(outs)

        devices = jax.devices()[:NCORES]
        mesh = Mesh(np.asarray(devices), ("core",))
        self.mesh = mesh
        in_specs = (PartitionSpec("core"),) * (self.n_params + n_outs)
        out_specs = (PartitionSpec("core"),) * n_outs
        self.fn = jax.jit(
            shard_map(_body, mesh=mesh, in_specs=in_specs,
                      out_specs=out_specs, check_rep=False),
            donate_argnums=donate, keep_unused=True)

    def __call__(self, in_maps):
        concat_in = [
            np.concatenate([np.asarray(in_maps[c][nm]) for c in range(NCORES)],
                           axis=0)
            for nm in self.in_names]
        concat_zeros = [
            np.zeros((NCORES * z.shape[0], *z.shape[1:]), z.dtype)
            for z in self.zero_outs]
        out_arrs = self.fn(*concat_in, *concat_zeros)
        return [
            {nm: np.asarray(out_arrs[i]).reshape(
                NCORES, *self.out_avals[i].shape)[c]
             for i, nm in enumerate(self.out_names)}
            for c in range(NCORES)]


_RUNNERS = {}


def _get_runner(reps: int, dbg: bool = False):
    key = (reps, dbg)
    if key not in _RUNNERS:
        _RUNNERS[key] = _PjrtRunner(_get_program(reps, dbg=dbg))
    return _RUNNERS[key]


def make_in_maps(x, W1, W2, Wq, bq, Wk, bk, Wv, bv, Wo, bo, W3):
    """Host-side sharding: slicing / transposition / constant assembly only."""
    f32, f16 = np.float32, np.float16
    x = np.asarray(x, f32)

    # Q mix: rows = the 11 reduced channels, cols = 16 Q outputs
    w12q = np.zeros((NCH, HID), f16)
    w12q[0:CH1, :] = np.asarray(W1, f32).T
    # K/(W3 V) mix: K -> cols 0..15; W3-folded V -> cols 32..34 (parked at
    # partition base 32 so its PE transpose is tile-position aligned);
    # col 35 = Z-ones row (filled by the bias plane)
    w3w2 = np.asarray(W3, f32) @ np.asarray(W2, f32)     # [3, 4]
    w12kv = np.zeros((NCH, NKV), f16)
    w12kv[CH1:CH1 + CH2, 0:HID] = np.asarray(W2, f32).T
    w12kv[CH1 + CH2:NCH, 2 * HID:2 * HID + CH1] = w3w2.T

    w3sum = np.asarray(W3, f32).sum(axis=1)              # [3]
    biasqkv = np.zeros((NKV, 2 * E), f32)
    biasqkv[0:HID, 0:E] = np.asarray(bq, f32)[None, :]
    biasqkv[0:HID, E:2 * E] = np.asarray(bk, f32)[None, :]
    biasqkv[2 * HID:2 * HID + CH1, E:2 * E] = (
        w3sum[:, None] * np.asarray(bv, f32)[None, :])
    biasqkv[NKV - 1, E:2 * E] = 1.0

    identt = np.zeros((NKV, CH1Z), f16)
    identt[0:CH1Z, :] = np.eye(CH1Z, dtype=f16)
    identt[2 * HID:NKV, :] = np.eye(CH1Z, dtype=f16)

    w3sr = np.tile(w3sum, B)[None, :].astype(f16)        # [1, 192], b*3+o

    in_maps = []
    for c in range(NCORES):
        sl = slice(c * TLOC, (c + 1) * TLOC)
        xt = np.ascontiguousarray(
            np.transpose(x[:, :, sl], (2, 1, 0)).reshape(TLOC, 7 * B))
        m = {
            "xt": xt.astype(f16),
            "xp": np.ascontiguousarray(
                x[:, :CH1, sl].reshape(CH1 * B, TLOC)).astype(f16),
            "wqkv": np.concatenate(
                [np.asarray(Wq, f32)[:, sl].T, np.asarray(Wk, f32)[:, sl].T,
                 np.asarray(Wv, f32)[:, sl].T], axis=1).astype(f16),
            "wot": np.asarray(Wo, f32)[sl, :].T.astype(f16),
            "bot": np.asarray(bo, f32)[sl][None, :].astype(f16),
            "w12q": w12q, "w12kv": w12kv, "biasqkv": biasqkv,
            "identt": identt, "w3sr": w3sr,
        }
        in_maps.append(m)
    return in_maps


def assemble_output(results):
    """[per-core yt [192, 2048]] -> [B, CH1, T]; row = b*CH1 + o."""
    arr = np.stack([res["yt"] for res in results], axis=0)  # [8, 192, 2048]
    return np.ascontiguousarray(
        arr.transpose(1, 0, 2).reshape(B, CH1, T))


def run(inputs, reps: int = 1, dbg: bool = False):
    runner = _get_runner(reps, dbg=dbg)
    in_maps = make_in_maps(**inputs)
    results = runner(in_maps)
    if dbg:
        return assemble_output(results), results
    return assemble_output(results)


def kernel(**inputs) -> np.ndarray:
    return run(inputs, reps=1)


def time_reps(inputs, reps: int, n: int = 10):
    """Per-call wall times with device-resident inputs (first call = warmup)."""
    import time
    import jax
    from jax.sharding import NamedSharding, PartitionSpec

    runner = _get_runner(reps)
    in_maps = make_in_maps(**inputs)
    concat = [
        np.concatenate([np.asarray(in_maps[c][nm]) for c in range(NCORES)],
                       axis=0)
        for nm in runner.in_names]
    sh = NamedSharding(runner.mesh, PartitionSpec("core"))
    dev = [jax.device_put(a, sh) for a in concat]
    times = []
    for i in range(n + 1):
        zeros = [np.zeros((NCORES * z.shape[0], *z.shape[1:]), z.dtype)
                 for z in runner.zero_outs]
        t0 = time.perf_counter()
        out = runner.fn(*dev, *zeros)
        jax.block_until_ready(out)
        dt = time.perf_counter() - t0
        if i > 0:
            times.append(dt)
    return times


# revision 9
# speedup vs baseline: 1.0432x; 1.0016x over previous
"""Trainium2 Bass kernel for nn_AttnResBlock (B=64, CH1=3, CH2=4, HID=16, T=16384, E=512).

Strategy: tensor-parallel split of the T dimension across 8 cores, fp16
datapath for every large tensor.

  y = p + W3 @ (attn(W1@p, W2@c | Wq,Wk,Wv) @ Wo)        p = x[:,:3], c = x[:,3:7]

The big weights (Wq/Wk/Wv [E,T], Wo [T,E]) dominate memory traffic.  Each
core owns a T-slice of 2048 and reads only its slice of each projection
weight; all large streams are fp16 (host-rounded), halving HBM bytes vs
fp32 while the PE accumulates in fp32 PSUM (matmul error ~1e-3 rel, well
inside the 2e-2 gate).

  stage A (T-parallel):   P[b,c,e]     = sum_{t in Ti} p[b,c,t] Wq[e,t]   (partial)
                          Ck/Cv[b,c,e] = sum_{t in Ti} c[b,c,t] W{k,v}[e,t]
    11 rows/batch (3 P + 4 Ck + 4 Cv) go through a fp16 ReduceScatter;
    core r receives batches 8r..8r+7.  Channel mixes and biases are applied
    AFTER the reduce (biases via a precomputed [49,E] bias plane added by
    DVE, so they ride along with the PSUM->SBUF eviction).
  stage B (B-parallel):   two block-diag matmuls produce Q [16,E] and K/V
                          [49,E] per batch (V parked at partition base 32 so
                          the V^T PE transposes are tile-position legal);
                          V^T via 4 tiny PE transposes (col 16 of each chunk
                          = 1.0 from the bias plane, which makes the attn
                          matmul also emit the softmax partition sums Z as
                          row 17); S^T = K^T Q, exp on ACT (fp16 out),
                          attn = V^T @ exp(S^T), W3-mix, 1/Z normalize.
                          Normalized attn3[e,(b,o)] is AllGather'ed
                          (tiny: 25KB/core).
  stage C (T-parallel):   y^T[(b,o), t] = at3^T Wo-slice contraction over e,
                          + bo*w3sum rank-1 term, + residual p (fp32 xp,
                          loaded during the collective window).

DMA instruction count is kept low (HWDGE dispatch costs ~625ns/DMA): weights
stream in 2-k-tile chunks, the ReduceScatter staging is 6 DMAs with
(ch,b)->(b,ch) row regrouping done by the DMA access pattern, and stage B
reads all of its post-scatter rows in a single DMA.
"""

import numpy as np

import concourse.bacc as bacc
import concourse.tile as tile
import concourse.mybir as mybir

F32 = mybir.dt.float32
F16 = mybir.dt.float16
AF = mybir.ActivationFunctionType
ALU = mybir.AluOpType

B, CH1, CH2, HID, T, E = 64, 3, 4, 16, 16384, 512
NCORES = 8
TLOC = T // NCORES          # 2048
KT = TLOC // 128            # 16 k-tiles in stage A
KCH = 4                     # k-tiles per DMA chunk
NCHUNK = KT // KCH          # 8 chunks
EC = E // 128               # 4 e/f chunks
BLOC = B // NCORES          # 8 batches per core in stage B
NCH = CH1 + CH2 + CH2       # 11 rows/batch through the ReduceScatter
NKV = 2 * HID + CH1 + 1     # 36 KV-mix rows: K 0..15, pad, W3V 32..34, Z 35
SCALE = 1.0 / np.sqrt(HID)  # attention scale
EXPSHIFT = 10.0             # exp(S*SCALE - 10): keeps exp inside fp16 range;
                            # softmax is shift-invariant so Z divides it out
CH1Z = CH1 + 1              # 3 W3-mixed channels + the softmax sum Z
GW = CH1Z * BLOC            # 32 AllGather columns per rank: (b, [o0 o1 o2 Z])


def build_program(reps: int = 1, dbg: bool = False):
    nc = bacc.Bacc("TRN2", target_bir_lowering=False, debug=False,
                   num_devices=NCORES)

    XW = 7 * B + 3 * E         # 1984 columns: [x (c,b) | wq | wk | wv]
    xw = nc.dram_tensor("xw", [TLOC, XW], F16, kind="ExternalInput")
    xp = nc.dram_tensor("xp", [CH1 * B, TLOC], F16, kind="ExternalInput")
    wot = nc.dram_tensor("wot", [E, TLOC], F16, kind="ExternalInput")
    bot = nc.dram_tensor("bot", [1, TLOC], F16, kind="ExternalInput")
    w12q = nc.dram_tensor("w12q", [NCH, HID], F16, kind="ExternalInput")
    w12kv = nc.dram_tensor("w12kv", [NCH, NKV], F16, kind="ExternalInput")
    biasqkv = nc.dram_tensor("biasqkv", [NKV, 2 * E], F32,
                             kind="ExternalInput")
    identt = nc.dram_tensor("identt", [NKV, CH1Z], F16,
                            kind="ExternalInput")
    w3sr = nc.dram_tensor("w3sr", [1, CH1 * B], F16, kind="ExternalInput")
    yt = nc.dram_tensor("yt", [CH1 * B, TLOC], F32, kind="ExternalOutput")

    rg = [list(range(NCORES))]

    with tile.TileContext(nc) as tc:
        with tc.tile_pool(name="const", bufs=1) as cp, \
             tc.tile_pool(name="dram", space="DRAM", bufs=1) as dp, \
             tc.tile_pool(name="xp", bufs=1) as xpool, \
             tc.tile_pool(name="wotp", bufs=1) as wotp:

            # small constants via the Pool (SWDGE) queue so they never
            # contend with the stage-A weight stream on HWDGE
            w12q_s = cp.tile([NCH, HID], F16, name="w12q_s")
            w12kv_s = cp.tile([NCH, NKV], F16, name="w12kv_s")
            biasqkv_s = cp.tile([NKV, 2 * E], F32, name="biasqkv_s")
            identt_s = cp.tile([NKV, CH1Z], F16, name="identt_s")
            w3sr_s = cp.tile([1, CH1 * B], F16, name="w3sr_s")
            bot_s = cp.tile([1, TLOC], F16, name="bot_s")
            nshift = cp.tile([128, 1], F32, name="nshift")
            nc.vector.memset(nshift[:], -EXPSHIFT)
            nc.gpsimd.dma_start(w12q_s[:], w12q[:])
            nc.gpsimd.dma_start(w12kv_s[:], w12kv[:])
            nc.gpsimd.dma_start(biasqkv_s[:], biasqkv[:])
            nc.gpsimd.dma_start(identt_s[:], identt[:])
            nc.gpsimd.dma_start(w3sr_s[:], w3sr[:])
            nc.gpsimd.dma_start(bot_s[:], bot[:])

            dbg_outs = {}
            if dbg:
                dbg_outs["dbg_rs"] = nc.dram_tensor(
                    "dbg_rs", [BLOC * NCH, E], F16, kind="ExternalOutput")
                dbg_outs["dbg_rsin"] = nc.dram_tensor(
                    "dbg_rsin", [B * NCH, E], F16, kind="ExternalOutput")
                dbg_outs["dbg_q"] = nc.dram_tensor(
                    "dbg_q", [HID, E], F16, kind="ExternalOutput")
                dbg_outs["dbg_kv"] = nc.dram_tensor(
                    "dbg_kv", [NKV, E], F16, kind="ExternalOutput")
                dbg_outs["dbg_et"] = nc.dram_tensor(
                    "dbg_et", [128, EC * E], F16, kind="ExternalOutput")
                dbg_outs["dbg_ag"] = nc.dram_tensor(
                    "dbg_ag", [NCORES * E, GW], F16, kind="ExternalOutput")
            for rep in range(reps):
                build_rep(nc, tc, dp, xpool, wotp, rep, rg, locals())

    nc.compile()
    return nc


def build_rep(nc, tc, dp, xpool, wotp, rep, rg, env):
    xw, xp, wot, yt, XW = (env[k] for k in
                           ("xw", "xp", "wot", "yt", "XW"))
    w12q_s, w12kv_s, biasqkv_s, identt_s = (
        env[k] for k in ("w12q_s", "w12kv_s", "biasqkv_s", "identt_s"))
    w3sr_s, bot_s, nshift = (env[k] for k in ("w3sr_s", "bot_s", "nshift"))
    dbg_outs = env.get("dbg_outs", {})

    r = f"r{rep}"

    # ---- DRAM bounce buffers for the collectives -------------------------
    rs_in = dp.tile([B * NCH, E], F16, name=f"rs_in_{r}", tag="rs_in", bufs=1)
    rs_out = dp.tile([BLOC * NCH, E], F16, name=f"rs_out_{r}", tag="rs_out",
                     bufs=1)
    ag_in = dp.tile([E, GW], F16, name=f"ag_in_{r}", tag="ag_in", bufs=1)
    ag_out = dp.tile([NCORES * E, GW], F16, name=f"ag_out_{r}", tag="ag_out",
                     bufs=1)

    # ---- stage A: big T-contraction ------------------------------------
    # x and the host-concatenated [wq|wk|wv] stream in k-tile chunks (two
    # HWDGE dispatches per chunk); small chunks first so the PE starts fast
    CHUNKS = (1, 1, 1, 1, 2, 2, 4, 4)
    assert sum(CHUNKS) == KT
    wws = []
    k0c = 0
    for c, kch in enumerate(CHUNKS):
        wc = xpool.tile([128, kch * XW], F16, name=f"xw{c}_{r}",
                        tag=f"xw{c}", bufs=1)
        rows = slice(k0c * 128, (k0c + kch) * 128)
        nc.sync.dma_start(
            wc[:].rearrange("p (a w) -> p a w", a=kch),
            xw[rows, :].rearrange("(a p) w -> p a w", a=kch))
        wws.append(wc)
        k0c += kch

    with tc.tile_pool(name="psA", space="PSUM", bufs=1) as psA, \
         tc.tile_pool(name="stgA", bufs=1) as sa:
        p0 = psA.tile([128, E], F32, name=f"p0_{r}", tag="p0", bufs=1)
        p1 = psA.tile([64, E], F32, name=f"p1_{r}", tag="p1", bufs=1)
        k0 = psA.tile([128, E], F32, name=f"k0_{r}", tag="k0", bufs=1)
        k1 = psA.tile([128, E], F32, name=f"k1_{r}", tag="k1", bufs=1)
        v0 = psA.tile([128, E], F32, name=f"v0_{r}", tag="v0", bufs=1)
        v1 = psA.tile([128, E], F32, name=f"v1_{r}", tag="v1", bufs=1)

        k0c = 0
        for c, kch in enumerate(CHUNKS):
            xc = wc = wws[c]
            for a in range(kch):
                k = k0c + a
                st, sp = (k == 0), (k == KT - 1)
                x0 = a * XW
                w0 = a * XW + 7 * B
                wq_a = wc[:, w0:w0 + E]
                wk_a = wc[:, w0 + E:w0 + 2 * E]
                wv_a = wc[:, w0 + 2 * E:w0 + 3 * E]
                nc.tensor.matmul(p0[:], xc[:, x0:x0 + 128], wq_a,
                                 start=st, stop=sp)
                nc.tensor.matmul(p1[:], xc[:, x0 + 128:x0 + 192], wq_a,
                                 start=st, stop=sp)
                nc.tensor.matmul(k0[:], xc[:, x0 + 192:x0 + 320], wk_a,
                                 start=st, stop=sp)
                nc.tensor.matmul(v0[:], xc[:, x0 + 192:x0 + 320], wv_a,
                                 start=st, stop=sp)
                nc.tensor.matmul(k1[:], xc[:, x0 + 320:x0 + 448], wk_a,
                                 start=st, stop=sp)
                nc.tensor.matmul(v1[:], xc[:, x0 + 320:x0 + 448], wv_a,
                                 start=st, stop=sp)
            k0c += kch

        # PSUM -> SBUF fp16, split across the two PSUM-capable engines.
        # p0s/k0s/v0s live in the rep-long xpool, sized [128, 2*TLOC] (only
        # [:, :E] used) so the wot/xpall tiles below reuse their buffers:
        # the tag-rotation WAR dependency delays those big loads until the
        # staging DMAs have read the evictions — i.e. into the collective
        # window — instead of stealing stage-A DMA bandwidth.
        p0s = xpool.tile([128, 2 * TLOC], F16, name=f"p0s_{r}", tag="p0s",
                         bufs=1)
        p1s = sa.tile([64, E], F16, name=f"p1s_{r}", tag="p1s", bufs=1)
        k0s = xpool.tile([128, 2 * TLOC], F16, name=f"k0s_{r}", tag="k0s",
                         bufs=1)
        k1s = sa.tile([128, E], F16, name=f"k1s_{r}", tag="k1s", bufs=1)
        v0s = xpool.tile([128, 2 * TLOC], F16, name=f"v0s_{r}", tag="v0s",
                         bufs=1)
        v1s = sa.tile([128, E], F16, name=f"v1s_{r}", tag="v1s", bufs=1)
        nc.vector.tensor_copy(p0s[:, 0:E], p0[:])
        nc.scalar.activation(p1s[:], p1[:], AF.Copy)
        nc.vector.tensor_copy(k0s[:, 0:E], k0[:])
        nc.scalar.activation(k1s[:], k1[:], AF.Copy)
        nc.vector.tensor_copy(v0s[:, 0:E], v0[:])
        nc.scalar.activation(v1s[:], v1[:], AF.Copy)

        # (ch,b) rows -> rs_in's (b,ch) rows, regrouped by the DMA pattern.
        # ch layout per batch: [P0 P1 P2 | Ck0..3 | Cv0..3].  The SBUF side
        # stays a plain 2-d partition walk (same element order as the 3-d
        # DRAM view; dma_start only requires equal sizes).
        rs_v = rs_in[:].rearrange("(b c) e -> c b e", c=NCH)
        for s_t, ch0, nch in ((p0s, 0, 2), (p1s, 2, 1), (k0s, 3, 2),
                              (k1s, 5, 2), (v0s, 7, 2), (v1s, 9, 2)):
            nc.sync.dma_start(rs_v[ch0:ch0 + nch], s_t[:, 0:E])

    nc.gpsimd.collective_compute(
        "ReduceScatter", ALU.add, replica_groups=rg,
        ins=[rs_in.opt()], outs=[rs_out.opt()],
    )
    if rep == 0 and "dbg_rs" in dbg_outs:
        nc.gpsimd.dma_start(dbg_outs["dbg_rs"][:], rs_out[:])
        nc.gpsimd.dma_start(dbg_outs["dbg_rsin"][:], rs_in[:])

    # all post-scatter rows in one DMA, channel-major so every per-batch
    # slice starts at partition 0 (engines cannot shift partition lanes);
    # emitted BEFORE the wot/xpall loads so it dispatches the moment the
    # ReduceScatter completes instead of queueing behind them
    rs_all = xpool.tile([NCH, BLOC * E], F16, name=f"rs_all_{r}",
                        tag="rs_all", bufs=1)
    nc.sync.dma_start(
        rs_all[:].rearrange("c (b e) -> c b e", b=BLOC),
        rs_out[:].rearrange("(b c) e -> c b e", c=NCH))

    # stage-C inputs, buffer-reusing the eviction tags (see above): the
    # WAR dependency releases them at the stage-A tail, so they stream
    # during the collective window
    wots = []
    for half, tag in ((0, "p0s"), (1, "k0s")):
        wo_t = xpool.tile([128, 2 * TLOC], F16, name=f"wo{half}_{r}",
                          tag=tag, bufs=1)
        nc.sync.dma_start(
            wo_t[:].rearrange("p (a t) -> p a t", a=2),
            wot[half * 256:(half + 1) * 256, :].rearrange(
                "(a p) t -> p a t", a=2))
        wots.append(wo_t)
    xpbig = xpool.tile([128, 2 * TLOC], F16, name=f"xpall_{r}", tag="v0s",
                       bufs=1)
    xpall = xpbig[0:96, :]
    nc.sync.dma_start(
        xpall.rearrange("p (m t) -> p m t", m=2),
        xp[:].rearrange("(m p) t -> p m t", m=2))

    # ---- stage B: per-batch attention ----------------------------------
    with tc.tile_pool(name="psB", space="PSUM", bufs=1) as psB, \
         tc.tile_pool(name="sbB", bufs=2) as sb:
        ag_s = sb.tile([128, EC * GW], F16, name=f"ag_s_{r}", tag="ag_s",
                       bufs=1)

        # per-batch state carried across the 1-batch software-pipeline skew
        qkv_sL = [None] * BLOC
        vt_sL = [None] * BLOC
        etL = [None] * BLOC
        apsL = [None] * BLOC

        def emit_mix(b):
            """channel mixes for batch b: Q and K/W3V/Z in separate PSUM
            tiles so the two DVE evictions (which also apply the biases)
            free their banks independently and pipeline with the consumers.
            qkv_s layout: columns 0..E-1 = Q (rows 0..15), E..2E-1 = K
            (rows 0..15) / W3V+ones (rows 32..35)."""
            rs_b = rs_all[:, b * E:(b + 1) * E]
            q_ps = psB.tile([HID, E], F32, name=f"qps{b}_{r}", tag="qps",
                            bufs=1)
            kv_ps = psB.tile([NKV, E], F32, name=f"kvps{b}_{r}", tag="kvps",
                             bufs=1)
            nc.tensor.matmul(q_ps[:], w12q_s[:], rs_b, start=True, stop=True)
            nc.tensor.matmul(kv_ps[:], w12kv_s[:], rs_b, start=True,
                             stop=True)
            qkv_s = sb.tile([NKV, 2 * E], F16, name=f"qkv_s{b}_{r}",
                            tag="qkv_s", bufs=2)
            nc.vector.tensor_tensor(out=qkv_s[0:HID, 0:E], in0=q_ps[:],
                                    in1=biasqkv_s[0:HID, 0:E], op=ALU.add)
            nc.vector.tensor_tensor(out=qkv_s[:, E:2 * E], in0=kv_ps[:],
                                    in1=biasqkv_s[:, E:2 * E], op=ALU.add)
            qkv_sL[b] = qkv_s

        def emit_vt(b):
            """(W3V | 1)^T: rows 32..35 of the K-half -> [128, 4] per
            f-chunk.  vtp and atp share PSUM banks (tag "tp")."""
            qkv_s = qkv_sL[b]
            vtp = psB.tile([128, EC * CH1Z], F16, name=f"vtp{b}_{r}",
                           tag="tp", bufs=1)
            for fc in range(EC):
                nc.tensor.transpose(
                    vtp[:, fc * CH1Z:(fc + 1) * CH1Z],
                    qkv_s[2 * HID:NKV, E + fc * 128:E + (fc + 1) * 128],
                    identt_s[2 * HID:NKV, :])
            vt_s = sb.tile([128, EC * CH1Z], F16, name=f"vt_s{b}_{r}",
                           tag="vt_s", bufs=2)
            nc.vector.tensor_copy(vt_s[:], vtp[:])
            vt_sL[b] = vt_s

        def emit_s(b):
            """four S^T chunk matmuls, one exp each (chunk granularity
            keeps the PE->ACT pipeline fine-grained)."""
            qkv_s = qkv_sL[b]
            q_ap = qkv_s[0:HID, 0:E]
            ets = []
            for fc in range(EC):
                sps = psB.tile([128, E], F32, name=f"sps{b}{fc}_{r}",
                               tag="s", bufs=3)
                nc.tensor.matmul(
                    sps[:], qkv_s[0:HID, E + fc * 128:E + (fc + 1) * 128],
                    q_ap, start=True, stop=True)
                et = sb.tile([128, E], F16, name=f"et{b}{fc}_{r}", tag="et",
                             bufs=8)
                nc.scalar.activation(et[:], sps[:], AF.Exp, scale=SCALE,
                                     bias=nshift[:])
                ets.append(et)
            etL[b] = ets

        def emit_attn(b):
            """attn matmuls for batch b (needs et[b] ready).

            Rows: 0..2 = W3-mixed attention (unnormalized), 3 = Z."""
            aps = psB.tile([CH1Z, E], F32, name=f"aps{b}_{r}", tag="attn",
                           bufs=1)
            vt_s, ets = vt_sL[b], etL[b]
            for fc in range(EC):
                nc.tensor.matmul(aps[:], vt_s[:, fc * CH1Z:(fc + 1) * CH1Z],
                                 ets[fc][:], start=(fc == 0), stop=(fc == 3))
            apsL[b] = aps

        def emit_back(b):
            """transpose attn rows into e-partition layout for the gather."""
            aps = apsL[b]
            an_s = sb.tile([CH1Z, E], F16, name=f"an_s{b}_{r}", tag="an_s",
                           bufs=2)
            nc.vector.tensor_copy(an_s[:], aps[:])
            atp = psB.tile([128, EC * CH1Z], F16, name=f"atp{b}_{r}",
                           tag="atp", bufs=1)
            for ec in range(EC):
                nc.tensor.transpose(
                    atp[:, ec * CH1Z:(ec + 1) * CH1Z],
                    an_s[:, ec * 128:(ec + 1) * 128],
                    identt_s[0:CH1Z, :])
            # ag_s column layout per e-chunk block: (b, [o0 o1 o2 Z])
            nc.vector.tensor_copy(
                ag_s[:].rearrange("p (c q) -> p c q", c=EC)[
                    :, :, b * CH1Z:(b + 1) * CH1Z],
                atp[:].rearrange("p (c k) -> p c k", c=EC))

        # software pipeline with a 1-batch skew: after each mix, the PE
        # chews on batch b-1 (attn + output transposes) while the DVE
        # evicts batch b's mixes, so no engine waits on the
        # mix->evict->S chain
        emit_mix(0)
        emit_vt(0)
        emit_s(0)
        for b in range(1, BLOC):
            emit_mix(b)
            emit_attn(b - 1)
            emit_back(b - 1)
            emit_vt(b)
            emit_s(b)
        emit_attn(BLOC - 1)
        emit_back(BLOC - 1)

        if rep == 0 and "dbg_q" in dbg_outs:
            nc.gpsimd.dma_start(dbg_outs["dbg_q"][:], qkv_sL[0][0:HID, 0:E])
            nc.gpsimd.dma_start(dbg_outs["dbg_kv"][:],
                                qkv_sL[0][:, E:2 * E])
        if rep == 0 and "dbg_et" in dbg_outs:
            nc.gpsimd.dma_start(dbg_outs["dbg_et"][:], etL[0][:])

        nc.sync.dma_start(
            ag_in[:].rearrange("(c p) w -> p c w", c=EC),
            ag_s[:].rearrange("p (c w) -> p c w", c=EC))

    nc.gpsimd.collective_compute(
        "AllGather", ALU.bypass, replica_groups=rg,
        ins=[ag_in.opt()], outs=[ag_out.opt()],
    )
    if rep == 0 and "dbg_ag" in dbg_outs:
        nc.gpsimd.dma_start(dbg_outs["dbg_ag"][:], ag_out[:])

    # ---- stage C: y^T[(b,o), t] = at3^T Wo^T + bias + residual ----------
    from concourse.bass import broadcast_tensor_aps

    with tc.tile_pool(name="psC", space="PSUM", bufs=1) as psC, \
         tc.tile_pool(name="sbC", bufs=1) as sc2:
        MH = CH1 * B // 2      # 96 (b,o) rows per M-half
        atall = sc2.tile([128, EC * NCORES * GW], F16, name=f"atall_{r}",
                         tag="atall", bufs=1)
        ag_v = ag_out[:].rearrange("(g c p) w -> c p g w", g=NCORES, c=EC)
        for ec in range(EC):
            nc.gpsimd.dma_start(
                atall[:, ec * NCORES * GW:(ec + 1) * NCORES * GW].rearrange(
                    "p (g w) -> p g w", g=NCORES),
                ag_v[ec])
        at3 = []
        for ec in range(EC):
            a_u = atall[:, ec * NCORES * GW:(ec + 1) * NCORES * GW]
            # normalize: at3[e, (g,b,o)] = att[e,(g,b,o)] * (1/Z[e,(g,b)])
            a_n = sc2.tile([128, CH1 * B], F16, name=f"at3{ec}_{r}",
                           tag=f"at3{ec}", bufs=1)
            u = a_u.rearrange("p (g b k) -> p g b k", g=NCORES, b=BLOC)
            zr = sc2.tile([128, B], F32, name=f"zr{ec}_{r}", tag=f"zr{ec}",
                          bufs=1)
            zr_v = zr[:].rearrange("p (g b one) -> p g b one", g=NCORES,
                                   one=1)
            nc.vector.reciprocal(zr_v, u[:, :, :, CH1:CH1Z])
            num, den = broadcast_tensor_aps(u[:, :, :, 0:CH1], zr_v)
            nc.vector.tensor_tensor(
                out=a_n[:].rearrange("p (g b k) -> p g b k", g=NCORES,
                                     b=BLOC),
                in0=num, in1=den, op=ALU.mult)
            at3.append(a_n)

        # yt row index is (b, o) = (g, w); M-halves split at g=4; each
        # half accumulates into one wide SBUF tile flushed by a single DMA
        for mh in range(2):
            c0 = mh * MH   # 0 or 96
            y_s = sc2.tile([MH, TLOC], F32, name=f"y_s{mh}_{r}",
                           tag=f"y_s{mh}", bufs=1)
            for m4 in range(EC):
                t0, t1 = m4 * 512, (m4 + 1) * 512
                yps = psC.tile([MH, 512], F32, name=f"yps{mh}{m4}_{r}",
                               tag="yps", bufs=8)
                # bias rank-1 first: it only needs constants, so it runs
                # during the AllGather window and keeps the PE warm
                nc.tensor.matmul(yps[:], w3sr_s[:, c0:c0 + MH],
                                 bot_s[:, t0:t1], start=True, stop=False)
                for ec in range(EC):
                    nc.tensor.matmul(
                        yps[:], at3[ec][:, c0:c0 + MH],
                        wots[ec // 2][:, (ec % 2) * TLOC + t0:
                                      (ec % 2) * TLOC + t1],
                        start=False, stop=(ec == EC - 1))
                nc.vector.tensor_tensor(
                    out=y_s[:, t0:t1], in0=yps[:],
                    in1=xpall[:, mh * TLOC + t0:mh * TLOC + t1], op=ALU.add)
                if m4 % 2 == 1:
                    # flush each 1024-column half-pair as soon as its adds
                    # land so the final write drains only 384KB
                    nc.sync.dma_start(
                        yt[c0:c0 + MH, m4 * 512 - 512:m4 * 512 + 512],
                        y_s[:, m4 * 512 - 512:m4 * 512 + 512])


_CACHE = {}


def _get_program(reps: int, dbg: bool = False):
    key = (reps, dbg)
    if key not in _CACHE:
        _CACHE[key] = build_program(reps, dbg=dbg)
    return _CACHE[key]


class _PjrtRunner:
    """jit-once wrapper around bass2jax so repeat calls skip recompile/reload."""

    def __init__(self, nc):
        import jax
        from jax.sharding import Mesh, PartitionSpec
        from jax.experimental.shard_map import shard_map
        from concourse import bass2jax

        bass2jax.install_neuronx_cc_hook()
        self.nc = nc
        in_names, out_names, out_avals, zero_outs = [], [], [], []
        partition_name = (nc.partition_id_tensor.name
                          if nc.partition_id_tensor else None)
        for alloc in nc.m.functions[0].allocations:
            if not isinstance(alloc, mybir.MemoryLocationSet):
                continue
            name = alloc.memorylocations[0].name
            if alloc.kind == "ExternalInput":
                if name != partition_name:
                    in_names.append(name)
            elif alloc.kind == "ExternalOutput":
                shape = tuple(alloc.tensor_shape)
                dtype = mybir.dt.np(alloc.dtype)
                out_names.append(name)
                out_avals.append(jax.core.ShapedArray(shape, dtype))
                zero_outs.append(np.zeros(shape, dtype))
        self.n_params = len(in_names)
        self.in_names = list(in_names)
        self.out_names = out_names
        self.out_avals = out_avals
        self.zero_outs = zero_outs
        all_in_names = in_names + out_names
        if partition_name is not None:
            all_in_names.append(partition_name)

        n_outs = len(out_names)
        donate = tuple(range(self.n_params, self.n_params + n_outs))

        def _body(*args):
            operands = list(args)
            if partition_name is not None:
                operands.append(bass2jax.partition_id_tensor())
            outs = bass2jax._bass_exec_p.bind(
                *operands,
                out_avals=tuple(out_avals),
                in_names=tuple(all_in_names),
                out_names=tuple(out_names),
                lowering_input_output_aliases=(),
                sim_require_finite=True,
                sim_require_nnan=True,
                nc=nc,
            )
            return tuple(outs)

        devices = jax.devices()[:NCORES]
        mesh = Mesh(np.asarray(devices), ("core",))
        self.mesh = mesh
        in_specs = (PartitionSpec("core"),) * (self.n_params + n_outs)
        out_specs = (PartitionSpec("core"),) * n_outs
        self.fn = jax.jit(
            shard_map(_body, mesh=mesh, in_specs=in_specs,
                      out_specs=out_specs, check_rep=False),
            donate_argnums=donate, keep_unused=True)

    def __call__(self, in_maps):
        concat_in = [
            np.concatenate([np.asarray(in_maps[c][nm]) for c in range(NCORES)],
                           axis=0)
            for nm in self.in_names]
        concat_zeros = [
            np.zeros((NCORES * z.shape[0], *z.shape[1:]), z.dtype)
            for z in self.zero_outs]
        out_arrs = self.fn(*concat_in, *concat_zeros)
        return [
            {nm: np.asarray(out_arrs[i]).reshape(
                NCORES, *self.out_avals[i].shape)[c]
             for i, nm in enumerate(self.out_names)}
            for c in range(NCORES)]


_RUNNERS = {}


def _get_runner(reps: int, dbg: bool = False):
    key = (reps, dbg)
    if key not in _RUNNERS:
        _RUNNERS[key] = _PjrtRunner(_get_program(reps, dbg=dbg))
    return _RUNNERS[key]


def make_in_maps(x, W1, W2, Wq, bq, Wk, bk, Wv, bv, Wo, bo, W3):
    """Host-side sharding: slicing / transposition / constant assembly only."""
    f32, f16 = np.float32, np.float16
    x = np.asarray(x, f32)

    # Q mix: rows = the 11 reduced channels, cols = 16 Q outputs
    w12q = np.zeros((NCH, HID), f16)
    w12q[0:CH1, :] = np.asarray(W1, f32).T
    # K/(W3 V) mix: K -> cols 0..15; W3-folded V -> cols 32..34 (parked at
    # partition base 32 so its PE transpose is tile-position aligned);
    # col 35 = Z-ones row (filled by the bias plane)
    w3w2 = np.asarray(W3, f32) @ np.asarray(W2, f32)     # [3, 4]
    w12kv = np.zeros((NCH, NKV), f16)
    w12kv[CH1:CH1 + CH2, 0:HID] = np.asarray(W2, f32).T
    w12kv[CH1 + CH2:NCH, 2 * HID:2 * HID + CH1] = w3w2.T

    w3sum = np.asarray(W3, f32).sum(axis=1)              # [3]
    biasqkv = np.zeros((NKV, 2 * E), f32)
    biasqkv[0:HID, 0:E] = np.asarray(bq, f32)[None, :]
    biasqkv[0:HID, E:2 * E] = np.asarray(bk, f32)[None, :]
    biasqkv[2 * HID:2 * HID + CH1, E:2 * E] = (
        w3sum[:, None] * np.asarray(bv, f32)[None, :])
    biasqkv[NKV - 1, E:2 * E] = 1.0

    identt = np.zeros((NKV, CH1Z), f16)
    identt[0:CH1Z, :] = np.eye(CH1Z, dtype=f16)
    identt[2 * HID:NKV, :] = np.eye(CH1Z, dtype=f16)

    w3sr = np.tile(w3sum, B)[None, :].astype(f16)        # [1, 192], b*3+o

    in_maps = []
    for c in range(NCORES):
        sl = slice(c * TLOC, (c + 1) * TLOC)
        xt = np.ascontiguousarray(
            np.transpose(x[:, :, sl], (2, 1, 0)).reshape(TLOC, 7 * B))
        m = {
            "xw": np.concatenate(
                [xt, np.asarray(Wq, f32)[:, sl].T,
                 np.asarray(Wk, f32)[:, sl].T,
                 np.asarray(Wv, f32)[:, sl].T], axis=1).astype(f16),
            "xp": np.ascontiguousarray(
                x[:, :CH1, sl].reshape(CH1 * B, TLOC)).astype(f16),
            "wot": np.asarray(Wo, f32)[sl, :].T.astype(f16),
            "bot": np.asarray(bo, f32)[sl][None, :].astype(f16),
            "w12q": w12q, "w12kv": w12kv, "biasqkv": biasqkv,
            "identt": identt, "w3sr": w3sr,
        }
        in_maps.append(m)
    return in_maps


def assemble_output(results):
    """[per-core yt [192, 2048]] -> [B, CH1, T]; row = b*CH1 + o."""
    arr = np.stack([res["yt"] for res in results], axis=0)  # [8, 192, 2048]
    return np.ascontiguousarray(
        arr.transpose(1, 0, 2).reshape(B, CH1, T))


def run(inputs, reps: int = 1, dbg: bool = False):
    runner = _get_runner(reps, dbg=dbg)
    in_maps = make_in_maps(**inputs)
    results = runner(in_maps)
    if dbg:
        return assemble_output(results), results
    return assemble_output(results)


def kernel(**inputs) -> np.ndarray:
    return run(inputs, reps=1)


def time_reps(inputs, reps: int, n: int = 10):
    """Per-call wall times with device-resident inputs (first call = warmup)."""
    import time
    import jax
    from jax.sharding import NamedSharding, PartitionSpec

    runner = _get_runner(reps)
    in_maps = make_in_maps(**inputs)
    concat = [
        np.concatenate([np.asarray(in_maps[c][nm]) for c in range(NCORES)],
                       axis=0)
        for nm in runner.in_names]
    sh = NamedSharding(runner.mesh, PartitionSpec("core"))
    dev = [jax.device_put(a, sh) for a in concat]
    times = []
    for i in range(n + 1):
        zeros = [np.zeros((NCORES * z.shape[0], *z.shape[1:]), z.dtype)
                 for z in runner.zero_outs]
        t0 = time.perf_counter()
        out = runner.fn(*dev, *zeros)
        jax.block_until_ready(out)
        dt = time.perf_counter() - t0
        if i > 0:
            times.append(dt)
    return times
